# revision 40
# baseline (speedup 1.0000x reference)
"""CRNN Trainium2 kernel: patchify-conv -> 3x3 conv -> pool -> GRU encoder ->
autoregressive GRU decoder. Pure data-parallel over batch (32 -> 8 cores x 4).

v3: composite conv. conv1 (patchify) and conv2 (3x3) have no nonlinearity
between them, so they fold into 9 tap weights W_t = W2_t @ W1 applied
directly to the fp8 patch buffer (gutter layout, zeros baked host-side).
Tap pairs are M-packed into the 128 PE output partitions with a constant
slot shift between the two halves (delta -8 rows / -1 col), so the whole
conv stack is 15 DoubleRow matmuls + 1 bias matmul per 8-frame group
(was 38). Epilogue combines the shifted PSUM halves with 3 DVE adds.
"""

import os
import sys

for _p in ("/opt/trn_rl_repo", "/root/.axon_site/_ro/trn_rl_repo"):
    if os.path.isdir(_p) and _p not in sys.path:
        sys.path.insert(0, _p)

import numpy as np

import concourse.bass as bass  # noqa: E402
import concourse.mybir as mybir  # noqa: E402
import concourse.tile as tile  # noqa: E402
from concourse import bacc  # noqa: E402
from concourse.bass_utils import run_bass_kernel_spmd  # noqa: E402

F32 = mybir.dt.float32
F8 = mybir.dt.float8e4
AF = mybir.ActivationFunctionType
ALU = mybir.AluOpType
DR = mybir.MatmulPerfMode.DoubleRow

# Model dims (hardcoded from the problem spec)
B, L, DS, DA, DC, DRN, DO, HOR = 32, 16, 12, 16, 64, 256, 2, 10
NCORES, BPC = 8, 4          # batch per core
NG, FPG = 8, 8              # 8 groups of 8 frames per core (frame idx = l*4+b)
BN_EPS = 1e-5
SC = 32.0                   # fp8 composite conv weight scale
LEAD, FW = 16, 16 + 8 * 64 + 16   # 544-col gutter row per patch K-chunk
# composite conv M-pack: P0 pairs (lower off, upper off) share delta -8;
# tap +9 runs solo into P0's lower half; P1 holds the delta -1 pair.
P0_PAIRS = [(-1, -9), (0, -8), (1, -7)]
P1_PAIR = (8, 7)
SOLO_TAP = 9

MM_DT_RNN = os.environ.get("BASS_MM_DT_RNN", "f16")


def _dt_of(tag):
    return {"f32": mybir.dt.float32, "f32r": mybir.dt.float32r,
            "bf16": mybir.dt.bfloat16, "f16": mybir.dt.float16}[tag]

LAST_EXEC_NS = None
LAST_RESULTS = None


def _layout(entries):
    """entries: (name, rows, width[, row0]) -> dict + total cols."""
    out = {}
    cols = 0
    for e in entries:
        name, rows, width = e[0], e[1], e[2]
        row0 = e[3] if len(e) > 3 else 0
        out[name] = (row0, rows, cols, width)
        cols += width
    return out, cols


# matmul operands (RNN matmul dtype)
SMM_LAYOUT, SMM_COLS = _layout([
    ("xt", 12, 64),          # per-core x transposed, col = l*4+b
    ("a0t", 12, 16),
    ("ait", 16, 16),
    ("anT", 80, 256),        # [an_w[:,16:80].T ; an_w[:,0:16].T] rows
    # composite conv bias: rank-10 basis x 0/1 tap-validity patterns
    ("bcT", 10, 64),         # [cnn1_b ; W2_t @ cnn_b per tap] rows
    ("bcP", 10, 512),        # per-slot validity patterns (tiled x8 frames)
    # gate bias-into-psum operands: one matmul per psum tile
    # (lhsT = bias chunks as rows, rhs = chunk->column selector)
    ("brz4", 4, 128),        # (b_ih+b_hh) rz chunks
    ("bni4", 4, 128),        # [b_hh n chunks ; b_ih n chunks] rows
    ("fib2", 2, 128),        # fi_b chunks
    ("sel16", 4, 16),        # selector: col n lights chunk n//4
    ("selni", 4, 16),        # interleaved: even col j -> bhh j//8,
                             # odd col j -> bih j//8
    ("sel8", 2, 8),
])
# bias/affine tables (always fp32)
SMB_LAYOUT, SMB_COLS = _layout([
    ("pscale", 64, 1),       # inv/49/SC
    ("pshift", 64, 1),
    ("a0b", 16, 1),
    ("aib", 16, 1),
    ("anb", 128, 2),         # an_b chunks as cols
    ("fnb", 2, 1),
])

def build_nc():
    nc = bacc.Bacc("TRN2", target_bir_lowering=False, debug=False,
                   num_devices=NCORES)
    mm_rnn = _dt_of(MM_DT_RNN)
    MR = mm_rnn

    h_fr = nc.dram_tensor("fr", [NG, 128, 6 * FW], F8, kind="ExternalInput")
    h_smm = nc.dram_tensor("smm", [128, SMM_COLS], MR, kind="ExternalInput")
    h_smb = nc.dram_tensor("smb", [128, SMB_COLS], F32, kind="ExternalInput")
    h_wc = nc.dram_tensor("wc", [128, 3456], F8, kind="ExternalInput")
    h_wih = nc.dram_tensor("wih", [128, 2 * 768], MR, kind="ExternalInput")
    h_whh = nc.dram_tensor("whh", [128, 2 * 768], MR, kind="ExternalInput")
    h_fi = nc.dram_tensor("fiw", [128, 2 * 256], MR, kind="ExternalInput")
    h_fn = nc.dram_tensor("fnw", [128, 4], MR, kind="ExternalInput")
    h_out = nc.dram_tensor("out", [2, 4 * HOR], F32, kind="ExternalOutput")

    def mm(out, lhsT, rhs, **kw):
        nc.tensor.matmul(out, lhsT, rhs, skip_group_check=True, **kw)

    with tile.TileContext(nc) as tc:
        from contextlib import ExitStack
        with ExitStack() as ctx:
            cpool = ctx.enter_context(tc.tile_pool(name="const", bufs=1))
            work = ctx.enter_context(tc.tile_pool(name="work", bufs=4))
            state = ctx.enter_context(tc.tile_pool(name="state", bufs=1))
            hpool = ctx.enter_context(tc.tile_pool(name="h", bufs=3))
            cps = ctx.enter_context(
                tc.tile_pool(name="cps", bufs=2, space="PSUM"))
            gps = ctx.enter_context(
                tc.tile_pool(name="gps", bufs=2, space="PSUM"))
            psr = ctx.enter_context(
                tc.tile_pool(name="psr", bufs=2, space="PSUM"))

            # ---- constants + frames to SBUF ----
            # Sync queue: g0 frames, conv weights, then the remaining
            # frame groups back-to-back (descriptor issue is ~0.6us each,
            # so one start per group, all resident). Scalar queue: the
            # small RNN weights, done well before first use.
            xins = []
            for g in range(NG):
                xin_g = cpool.tile([128, 6, FW], F8, tag=f"xin{g}")
                xins.append(xin_g)
            nc.sync.dma_start(
                xins[0][:].rearrange("p a b -> p (a b)"), h_fr[0])
            wc = cpool.tile([128, 3456], F8, tag="wc")
            nc.sync.dma_start(wc[:], h_wc[:])
            for g in range(1, NG):
                nc.sync.dma_start(
                    xins[g][:].rearrange("p a b -> p (a b)"), h_fr[g])
            smm = cpool.tile([128, SMM_COLS], MR, tag="smm")
            nc.scalar.dma_start(smm[:], h_smm[:])
            smb = cpool.tile([128, SMB_COLS], F32, tag="smb")
            nc.scalar.dma_start(smb[:], h_smb[:])
            wih = cpool.tile([128, 2 * 768], MR, tag="wih")
            nc.scalar.dma_start(wih[:], h_wih[:])
            whh = cpool.tile([128, 2 * 768], MR, tag="whh")
            nc.scalar.dma_start(whh[:], h_whh[:])
            fiw = cpool.tile([128, 2 * 256], MR, tag="fiw")
            nc.scalar.dma_start(fiw[:], h_fi[:])
            fnw = cpool.tile([128, 4], MR, tag="fnw")
            nc.scalar.dma_start(fnw[:], h_fn[:])

            def sv(name):  # matmul-operand view (RNN dtype)
                r0, rows, off, width = SMM_LAYOUT[name]
                return smm[r0:r0 + rows, off:off + width]

            def svc(name, c0, w):
                r0, rows, off, width = SMM_LAYOUT[name]
                assert c0 + w <= width
                return smm[r0:r0 + rows, off + c0:off + c0 + w]

            def svf(name):  # fp32 bias/affine view
                r0, rows, off, width = SMB_LAYOUT[name]
                return smb[r0:r0 + rows, off:off + width]

            def svcf(name, c0, w):
                r0, rows, off, width = SMB_LAYOUT[name]
                assert c0 + w <= width
                return smb[r0:r0 + rows, off + c0:off + c0 + w]

            # PE warm-up: the tensor engine p-state ramps only under load,
            # and the first real matmuls otherwise run ~2.5x slow. Burn the
            # DMA-wait window (~2.5-10us) with throwaway matmuls on a
            # zeroed tile.
            wtile = work.tile([128, 512], F8, tag="warm")
            nc.vector.memset(wtile[:], 0.0)
            pw = psr.tile([64, 512], F32, tag="ps")
            for wi in range(16):
                mm(pw[:], wtile[:, 0:64], wtile[:, 0:512],
                   start=True, stop=True)

            # ---- persistent state tiles ----
            s2 = state.tile([16, 64], MR, tag="s2")
            s_enc = state.tile([128, 2, 64], MR, tag="senc")
            preds = state.tile([2, 4 * HOR], F32, tag="preds")
            # GRU scan operand tiles (even slots stay zero forever; odd
            # slots rewritten each step). Layout: slot 2i(+1) with pair
            # i = (chunk i//4, batch i%4) matching gate psum column order.
            rzscan = state.tile([128, 32], F32, tag="rzscan")
            nc.vector.memset(rzscan[:], 0.0)
            ozscan = state.tile([128, 16], F32, tag="ozscan")
            nc.gpsimd.memset(ozscan[:], 0.0)
            tzscan = state.tile([128, 16], F32, tag="tzscan")
            nc.gpsimd.memset(tzscan[:], 0.0)
            rz2 = rzscan[:].rearrange("p (a b) -> p a b", b=2)
            oz2 = ozscan[:].rearrange("p (a b) -> p a b", b=2)
            tz2 = tzscan[:].rearrange("p (a b) -> p a b", b=2)

            # ---- state adapters: s1 = relu(a0 x); s2 = s1 + relu(ai s1) ----
            # Emitted mid-group-0 so the conv1 matmuls (which only need
            # w1+xin0) lead the tensor queue instead of stalling on smm.
            def emit_adapters():
                pa = psr.tile([16, 64], F32, tag="ps")
                mm(pa[:], sv("a0t"), sv("xt"), start=True, stop=True)
                s1 = work.tile([16, 64], MR, tag="s1")
                nc.scalar.activation(s1[:], pa[:], AF.Relu, bias=svf("a0b"))
                pb = psr.tile([16, 64], F32, tag="ps")
                mm(pb[:], sv("ait"), s1[:], start=True, stop=True)
                s1b = work.tile([16, 64], MR, tag="s1")
                nc.scalar.activation(s1b[:], pb[:], AF.Relu, bias=svf("aib"))
                nc.vector.tensor_add(s2[:], s1[:], s1b[:])

            # encoder hidden state: odd slots of a scan-layout tile
            h0 = hpool.tile([128, 16], MR, tag="h")
            nc.gpsimd.memset(h0[:], 0.0)
            h_cur = h0[:].rearrange("p (a b) -> p a b", b=2)[:, :, 1]

            def whh_mms(prz16, pni16, hv):
                """whh gate matmuls for one step (the only mms after h)."""
                for mc in range(4):
                    reg = prz16[:, mc * 4:(mc + 1) * 4]
                    for kc in range(2):
                        mm(reg, whh[:, kc * 768 + mc * 128:
                                    kc * 768 + (mc + 1) * 128],
                           hv[:, kc * 4:(kc + 1) * 4],
                           start=False, stop=(mc == 3 and kc == 1))
                pniv = pni16.rearrange("p (c k) -> p c k", c=2)
                for mc2 in range(2):
                    reg = pniv[:, mc2, 0:8:2]
                    for kc in range(2):
                        mm(reg, whh[:, kc * 768 + (4 + mc2) * 128:
                                    kc * 768 + (5 + mc2) * 128],
                           hv[:, kc * 4:(kc + 1) * 4],
                           start=False, stop=(mc2 == 1 and kc == 1))

            def cell_chain(prz16, pni16, hv, after_sigma=None):
                """sigma -> scan(r*ghn+gin) -> tanh -> scan((1-z)n+zh).
                Returns the new hidden as an odd-slot view."""
                nc.scalar.activation(rz2[:, :, 1], prz16, AF.Sigmoid)
                if after_sigma is not None:
                    after_sigma()
                zv = rz2[:, 8:16, 1]
                nc.gpsimd.tensor_scalar(oz2[:, :, 1], zv, -1.0, 1.0,
                                        op0=ALU.mult, op1=ALU.add)
                nc.gpsimd.tensor_mul(tz2[:, :, 1], zv, hv)
                ns = work.tile([128, 16], F32, tag="nscan")
                nc.vector.tensor_tensor_scan(ns[:], rzscan[:, 0:16], pni16,
                                             0.0, op0=ALU.mult, op1=ALU.add)
                nc.scalar.activation(
                    tz2[:, :, 0],
                    ns[:].rearrange("p (a b) -> p a b", b=2)[:, :, 1],
                    AF.Tanh)
                hs = hpool.tile([128, 16], MR, tag="h")
                nc.vector.tensor_tensor_scan(hs[:], ozscan[:], tzscan[:],
                                             0.0, op0=ALU.mult, op1=ALU.add)
                return hs[:].rearrange("p (a b) -> p a b", b=2)[:, :, 1]

            def enc_preload(g):
                """Per-group gate psum tiles for steps 2g, 2g+1: biases +
                wih @ s_enc land before h is even known."""
                # start=True only on the bank's first mm: a start marks the
                # WHOLE psum bank pending-zero, so later first-writes of
                # other regions zero-fill implicitly (start=False).
                ep = gps.tile([128, 5, 16], F32, tag="eg")
                prz, pni = ep[:, 0:2, :], ep[:, 2:4, :]
                for ti in range(2):
                    mm(prz[:, ti, :], sv("brz4"), sv("sel16"),
                       start=(ti == 0), stop=False)
                    mm(pni[:, ti, :], sv("bni4"), sv("selni"),
                       start=False, stop=False)
                # pad write: clears the 16 elements past pni so CoreSim's
                # strided zero-region window never sees mixed state
                mm(ep[:, 4, :], sv("brz4"), sv("sel16"),
                   start=False, stop=True)
                pniv = pni.rearrange("p s (c k) -> p s c k", c=2)
                for ti in range(2):
                    xs = [s_enc[:, kc, g * FPG + ti * 4:g * FPG + ti * 4 + 4]
                          for kc in range(2)]
                    for mc in range(4):
                        reg = prz[:, ti, mc * 4:(mc + 1) * 4]
                        for kc in range(2):
                            mm(reg, wih[:, kc * 768 + mc * 128:
                                        kc * 768 + (mc + 1) * 128],
                               xs[kc], start=False, stop=False)
                    for mc2 in range(2):
                        reg = pniv[:, ti, mc2, 1:8:2]
                        for kc in range(2):
                            mm(reg, wih[:, kc * 768 + (4 + mc2) * 128:
                                        kc * 768 + (5 + mc2) * 128],
                               xs[kc], start=False,
                               stop=(mc2 == 1 and kc == 1))
                return prz, pni

            # ---- composite conv + features, per group of 8 frames ----
            for g in range(NG):
                xin = xins[g]
                pc = cps.tile([128, 2, 512], F32, tag="cps")
                P0, P1 = pc[:, 0, :], pc[:, 1, :]

                def rhs(off, q):
                    a = LEAD + off
                    return xin[:, 2 * q:2 * q + 2, a:a + 512]

                def wcv(blk, m):  # block at col 256*?: [128, 2, m]
                    return wc[:, blk:blk + 2 * m].rearrange(
                        "p (a m) -> p a m", a=2)

                # P0: first pair mm starts the accumulation over all 128
                # rows; bias + solo-tap (M=64) mms ride in the middle; the
                # last pair mm closes the group.
                for p, (lo, up) in enumerate(P0_PAIRS):
                    for q in range(3):
                        first = (p == 0 and q == 0)
                        last = (p == 2 and q == 2)
                        mm(P0, wcv((p * 3 + q) * 256, 128), rhs(lo, q),
                           start=first, stop=last, perf_mode=DR)
                        if first:
                            # conv bias (rank-10 basis x validity patterns)
                            mm(P0[0:64], sv("bcT"), sv("bcP"),
                               start=False, stop=False)
                            for q2 in range(3):
                                mm(P0[0:64], wcv(2304 + q2 * 128, 64),
                                   rhs(SOLO_TAP, q2),
                                   start=False, stop=False, perf_mode=DR)
                for q in range(3):
                    mm(P1, wcv(2688 + q * 256, 128), rhs(P1_PAIR[0], q),
                       start=(q == 0), stop=(q == 2), perf_mode=DR)

                if g == 0:
                    emit_adapters()

                # epilogue: combine shifted psum halves -> relu -> sum ->
                # affine. upper halves hold the paired tap accumulated at
                # slot+delta (P0 delta -8 = one grid row, P1 delta -1).
                p0g = pc[0:64, 0, :].rearrange("p (f a b) -> p f a b",
                                               a=8, b=8)
                p0u = pc[64:128, 0, :].rearrange("p (f a b) -> p f a b",
                                                 a=8, b=8)
                p1g = pc[0:64, 1, :].rearrange("p (f a b) -> p f a b",
                                               a=8, b=8)
                p1u = pc[64:128, 1, :].rearrange("p (f a b) -> p f a b",
                                                 a=8, b=8)
                # DVE/ACT ops cannot read two PSUM operands in one
                # instruction: stage the upper halves through SBUF.
                u0 = work.tile([64, 8, 7, 7], F32, tag="epu0")
                nc.scalar.activation(u0[:], p0u[:, :, 0:7, 1:8], AF.Copy)
                u1 = work.tile([64, 8, 7, 7], F32, tag="epu1")
                nc.vector.tensor_copy(u1[:], p1u[:, :, 1:8, 0:7])
                t0 = work.tile([64, 8, 7, 7], F32, tag="ep0")
                nc.vector.tensor_add(t0[:], p0g[:, :, 1:8, 1:8], u0[:])
                t1 = work.tile([64, 8, 7, 7], F32, tag="ep1")
                nc.vector.tensor_add(t1[:], p1g[:, :, 1:8, 1:8], u1[:])
                t2 = work.tile([64, 8, 7, 7], F32, tag="ep2")
                nc.vector.tensor_add(t2[:], t0[:], t1[:])
                t3 = work.tile([64, 8, 7, 7], F32, tag="ep")
                nc.vector.tensor_scalar_max(t3[:], t2[:], 0.0)
                red = work.tile([64, 8], F32, tag="red")
                nc.vector.tensor_reduce(red[:], t3[:],
                                        axis=mybir.AxisListType.XY,
                                        op=ALU.add)
                feats = work.tile([80, 8], MR, tag="feats")
                nc.scalar.activation(feats[0:64, :], red[:], AF.Identity,
                                     bias=svf("pshift"), scale=svf("pscale"))

                # an: relu(an_w [s2; feats] + an_b), one K=80 matmul per half
                gcol = slice(g * FPG, (g + 1) * FPG)
                nc.gpsimd.tensor_copy(feats[64:80, :], s2[:, gcol])
                for mc in range(2):
                    pan = psr.tile([128, FPG], F32, tag="ps")
                    mm(pan[:], svc("anT", mc * 128, 128), feats[:],
                       start=True, stop=True)
                    nc.scalar.activation(s_enc[:, mc, gcol], pan[:], AF.Relu,
                                         bias=svcf("anb", mc, 1))

                # encoder steps that become ready after this group
                eprz, epni = enc_preload(g)
                for ti in range(2):
                    whh_mms(eprz[:, ti, :], epni[:, ti, :], h_cur)
                    h_cur = cell_chain(eprz[:, ti, :], epni[:, ti, :], h_cur)

            # ---- decoder ----
            # Emission order puts everything that depends only on hn(t-1)
            # (whh parts) ahead of the xr(t-1)-dependent wih work, and
            # defers fn(t-1) behind the whh block, so the tensor queue
            # keeps moving during the fi/xr window.
            def emit_fn(x, tt):
                pfn = psr.tile([2, 4], F32, tag="ps")
                for kc in range(2):
                    mm(pfn[:], fnw[:, kc * 2:(kc + 1) * 2],
                       x[:, kc * 4:(kc + 1) * 4],
                       start=(kc == 0), stop=(kc == 1))
                nc.scalar.activation(preds[:, tt * 4:(tt + 1) * 4], pfn[:],
                                     AF.Tanh, bias=svf("fnb"))

            xi, hh = h_cur, h_cur
            for t in range(HOR):
                dp = gps.tile([128, 5, 16], F32, tag="eg")
                prz, pni = dp[:, 0, :], dp[:, 1, :]
                mm(prz, sv("brz4"), sv("sel16"), start=True, stop=False)
                mm(pni, sv("bni4"), sv("selni"), start=False, stop=False)
                pniv = pni.rearrange("p (c k) -> p c k", c=2)
                # whh parts (ready at hh)
                for mc in range(4):
                    reg = prz[:, mc * 4:(mc + 1) * 4]
                    for kc in range(2):
                        mm(reg, whh[:, kc * 768 + mc * 128:
                                    kc * 768 + (mc + 1) * 128],
                           hh[:, kc * 4:(kc + 1) * 4],
                           start=False, stop=False)
                for mc2 in range(2):
                    reg = pniv[:, mc2, 0:8:2]
                    for kc in range(2):
                        mm(reg, whh[:, kc * 768 + (4 + mc2) * 128:
                                    kc * 768 + (5 + mc2) * 128],
                           hh[:, kc * 4:(kc + 1) * 4],
                           start=False, stop=False)
                if t > 0:
                    emit_fn(xi, t - 1)
                # rz wih parts (ready at xi); sigma waits only these
                for mc in range(4):
                    reg = prz[:, mc * 4:(mc + 1) * 4]
                    for kc in range(2):
                        mm(reg, wih[:, kc * 768 + mc * 128:
                                    kc * 768 + (mc + 1) * 128],
                           xi[:, kc * 4:(kc + 1) * 4],
                           start=False, stop=(mc == 3 and kc == 1))
                # fi bias rides the idle tensor window before hn is ready
                pfi = psr.tile([128, 8], F32, tag="ps")
                mm(pfi[:], sv("fib2"), sv("sel8"), start=True, stop=False)

                def wih_odds(xiv=xi, pv=pniv, dpv=dp):
                    # n-gate wih + pad: emitted after sigma so it does not
                    # wait on them (scan1 does). Pad first: it must clear
                    # the bytes the odds' zero-region windows overrun into.
                    mm(dpv[:, 2, :], sv("brz4"), sv("sel16"),
                       start=False, stop=True)
                    for mc2 in range(2):
                        reg = pv[:, mc2, 1:8:2]
                        for kc in range(2):
                            mm(reg, wih[:, kc * 768 + (4 + mc2) * 128:
                                        kc * 768 + (5 + mc2) * 128],
                               xiv[:, kc * 4:(kc + 1) * 4],
                               start=False, stop=(mc2 == 1 and kc == 1))

                hn = cell_chain(prz, pni, hh, after_sigma=wih_odds)

                # final_i residual: xr = hn + relu(fi hn + fi_b)
                for mc2 in range(2):
                    reg = pfi[:, mc2 * 4:(mc2 + 1) * 4]
                    for kc2 in range(2):
                        mm(reg, fiw[:, kc2 * 256 + mc2 * 128:
                                    kc2 * 256 + (mc2 + 1) * 128],
                           hn[:, kc2 * 4:(kc2 + 1) * 4],
                           start=False, stop=(mc2 == 1 and kc2 == 1))
                xr = hpool.tile([128, 8], MR, tag="xr")
                nc.vector.scalar_tensor_tensor(
                    xr[:], pfi[:], 0.0, hn,
                    op0=ALU.max, op1=ALU.add)
                xi, hh = xr[:], hn
            emit_fn(xi, HOR - 1)

            nc.sync.dma_start(h_out[:], preds[:])

    nc.finalize()
    return nc


# ---------------- host-side data prep ----------------

def _prep_frames(frames):
    """frames (32,16,3,112,112) -> per-core [NG, 128, 6*FW] gutter-layout
    patch-T fp8 (8x8 cell grid per frame, row0/col0 + LEAD/TAIL zeros)."""
    out = np.empty((NCORES, NG, 128, 6 * FW), mybir.dt.np(F8))
    fr = np.ascontiguousarray(frames, np.float32)
    for c in range(NCORES):
        fb = fr[c * BPC:(c + 1) * BPC]  # (4, 16, 3, 112, 112)
        a = fb.reshape(BPC, L, 3, 7, 16, 7, 16)
        # -> [l, b, ch, kh, kw, ph, pw]
        a = a.transpose(1, 0, 2, 4, 6, 3, 5)
        a = a.reshape(L, BPC, 768, 49)
        a = a.reshape(NG, 2, BPC, 6, 128, 49)
        # -> [g, k, p, li, b, s]
        a = a.transpose(0, 3, 4, 1, 2, 5)
        a = a.reshape(NG, 6, 128, 8, 7, 7)
        buf = np.zeros((NG, 6, 128, FW), np.float32)
        grid = buf[:, :, :, LEAD:LEAD + 512].reshape(NG, 6, 128, 8, 8, 8)
        grid[:, :, :, :, 1:8, 1:8] = a
        out[c] = buf.transpose(0, 2, 1, 3).reshape(
            NG, 128, 6 * FW).astype(mybir.dt.np(F8))
    return out


def _tap_weights(iv):
    """Composite per-tap weights W_t = cnn1_w[:,:,dh,dw] @ W1 (64, 768),
    keyed by gutter-slot offset (dh-1)*8 + (dw-1), scaled by SC."""
    W1f = iv["cnn_w"].reshape(576, 768).astype(np.float32)
    T = {}
    for dh in range(3):
        for dw in range(3):
            off = (dh - 1) * 8 + (dw - 1)
            T[off] = (iv["cnn1_w"][:, :, dh, dw].astype(np.float32)
                      @ W1f) * SC
    return T


def _prep_weights(iv):
    w = {}
    f8 = mybir.dt.np(F8)
    T = _tap_weights(iv)

    # composite conv lhsT blocks, in matmul emission order:
    # 9x [128, 2, 128] P0 pair blocks, 3x [128, 2, 64] solo-tap blocks,
    # 3x [128, 2, 128] P1 pair blocks.
    wcb = np.zeros((128, 3456), np.float32)
    col = 0
    for pair in P0_PAIRS + [None, P1_PAIR]:
        for q in range(3):
            if pair is None:
                blk = np.zeros((128, 2, 64), np.float32)
                for j in range(2):
                    c = 2 * q + j
                    blk[:, j, :] = T[SOLO_TAP][:, c * 128:(c + 1) * 128].T
                wcb[:, col:col + 128] = blk.reshape(128, 128)
                col += 128
            else:
                lo, up = pair
                blk = np.zeros((128, 2, 128), np.float32)
                for j in range(2):
                    c = 2 * q + j
                    blk[:, j, 0:64] = T[lo][:, c * 128:(c + 1) * 128].T
                    blk[:, j, 64:128] = T[up][:, c * 128:(c + 1) * 128].T
                wcb[:, col:col + 256] = blk.reshape(128, 256)
                col += 256
    assert col == 3456
    w["wc"] = np.ascontiguousarray(wcb).astype(f8)

    rdt = mybir.dt.np(_dt_of(MM_DT_RNN))
    for name, key in (("wih", "w_ih"), ("whh", "w_hh")):
        T = iv[key].T.astype(np.float32)  # (256, 768)
        w[name] = np.ascontiguousarray(
            T.reshape(2, 128, 768).transpose(1, 0, 2).reshape(
                128, 1536)).astype(rdt)
    T = iv["fi_w"].T.astype(np.float32)  # (256, 256)
    w["fiw"] = np.ascontiguousarray(
        T.reshape(2, 128, 256).transpose(1, 0, 2).reshape(128, 512)).astype(rdt)
    T = iv["fn_w"].T.astype(np.float32)  # (256, 2)
    w["fnw"] = np.ascontiguousarray(
        T.reshape(2, 128, 2).transpose(1, 0, 2).reshape(128, 4)).astype(rdt)
    return w


def _prep_smalls(iv, x, core):
    smm = np.zeros((128, SMM_COLS), mybir.dt.np(_dt_of(MM_DT_RNN)))
    smb = np.zeros((128, SMB_COLS), np.float32)

    def put(name, arr):
        if name in SMM_LAYOUT:
            r0, rows, off, width = SMM_LAYOUT[name]
            dst = smm
        else:
            r0, rows, off, width = SMB_LAYOUT[name]
            dst = smb
        a = np.asarray(arr, np.float32).reshape(rows, width)
        dst[r0:r0 + rows, off:off + width] = a.astype(dst.dtype)

    # composite conv bias = cnn1_b + sum over in-range taps of
    # (W2_t @ cnn_b): rank-10 basis (bcT) x 0/1 validity patterns (bcP),
    # folded into PSUM by one matmul; x SC to match the psum scale
    M = np.einsum("oiab,i->oab", iv["cnn1_w"], iv["cnn_b"]).astype(np.float32)
    bct = np.zeros((10, 64), np.float32)
    bcp = np.zeros((10, 512), np.float32)
    bct[0] = iv["cnn1_b"].astype(np.float32)
    bcp[0] = 1.0
    grid = bcp.reshape(10, 8, 8, 8)
    ti = 1
    for dh in range(3):
        for dw in range(3):
            bct[ti] = M[:, dh, dw]
            for r in range(8):
                for cc in range(8):
                    if 2 <= r + dh <= 8 and 2 <= cc + dw <= 8:
                        grid[ti, :, r, cc] = 1.0
            ti += 1
    put("bcT", bct * SC)
    put("bcP", bcp)

    inv = iv["bn_g"] / np.sqrt(iv["bn_v"] + BN_EPS)
    put("pscale", (inv / 49.0 / SC)[:, None])
    put("pshift", (iv["bn_b"] - iv["bn_m"] * inv)[:, None])

    xb = x[core * BPC:(core + 1) * BPC]  # (4, 16, 12)
    put("xt", xb.transpose(2, 1, 0).reshape(12, 64))

    put("a0t", iv["a0_w"].T)
    put("a0b", iv["a0_b"][:, None])
    put("ait", iv["ai_w"].T)
    put("aib", iv["ai_b"][:, None])
    put("anT", np.concatenate([iv["an_w"][:, 16:80].T,
                               iv["an_w"][:, 0:16].T], axis=0))
    put("anb", iv["an_b"].reshape(2, 128).T)

    put("fnb", iv["fn_b"][:, None])
    bs = (iv["b_ih"] + iv["b_hh"]).astype(np.float32)
    put("brz4", bs[:512].reshape(4, 128))
    # n-gate interleaved bias: rows [bhh c0, bhh c1, bih c0, bih c1];
    # slot j = c*8 + b*2 + parity -> row parity*2 + c
    put("bni4", np.concatenate([iv["b_hh"][512:].reshape(2, 128),
                                iv["b_ih"][512:].reshape(2, 128)]))
    selni = np.zeros((4, 16), np.float32)
    for j in range(16):
        selni[(j % 2) * 2 + j // 8, j] = 1.0
    put("selni", selni)
    put("fib2", iv["fi_b"].reshape(2, 128))
    put("sel16", np.repeat(np.eye(4, dtype=np.float32), 4, axis=1))
    put("sel8", np.repeat(np.eye(2, dtype=np.float32), 4, axis=1))
    return smm, smb


def make_in_maps(inputs):
    iv = {k: np.asarray(v, np.float32) for k, v in inputs.items()}
    frames = iv["frames"]
    x = iv["x"]
    fr_all = _prep_frames(frames)
    w = _prep_weights(iv)
    in_maps = []
    for c in range(NCORES):
        smm, smb = _prep_smalls(iv, x, c)
        m = {"fr": np.ascontiguousarray(fr_all[c]), "smm": smm, "smb": smb}
        m.update(w)
        in_maps.append(m)
    return in_maps


_NC_CACHE = None


def get_nc():
    global _NC_CACHE
    if _NC_CACHE is None:
        _NC_CACHE = build_nc()
    return _NC_CACHE


def _install_ntff_hook():
    """The agent image's antenv lacks axon_hooks; synthesize it so
    run_bass_kernel_spmd(trace=True) can capture NTFF profiles."""
    try:
        from antenv.axon_hooks import get_axon_ntff_profile_hook  # noqa: F401
        return True
    except ImportError:
        pass
    try:
        import types
        import antenv
        if "/root/.axon_site" not in sys.path:
            sys.path.insert(0, "/root/.axon_site")
        from trn_agent_boot.trn_boot import _ntff_profile_via_ctypes
        hook = _ntff_profile_via_ctypes("/opt/axon/libaxon_pjrt.so")
        mod = types.ModuleType("antenv.axon_hooks")
        mod.get_axon_ntff_profile_hook = lambda: hook
        mod.set_axon_ntff_profile_hook = lambda h: None
        sys.modules["antenv.axon_hooks"] = mod
        antenv.axon_hooks = mod
        return hook is not None
    except Exception as e:  # pragma: no cover - profiling is best-effort
        print(f"ntff hook install failed: {e}")
        return False


def kernel(**inputs):
    global LAST_EXEC_NS, LAST_RESULTS
    nc = get_nc()
    in_maps = make_in_maps(inputs)
    trace = bool(int(os.environ.get("KERNEL_TRACE", "0")))
    if trace:
        trace = _install_ntff_hook()
    res = run_bass_kernel_spmd(nc, in_maps, core_ids=list(range(NCORES)),
                               trace=trace)
    LAST_RESULTS = res
    LAST_EXEC_NS = res.exec_time_ns
    outs = []
    for c in range(NCORES):
        o = res.results[c]["out"]  # (2, 40)
        outs.append(o.reshape(2, HOR, BPC).transpose(1, 2, 0)[:, :, None, :])
    return np.concatenate(outs, axis=1).astype(np.float32)


if __name__ == "__main__":
    nc = get_nc()
    print("built ok; instructions:",
          sum(len(bb.instructions) for bb in nc.main_func.blocks))



# revision 45
# speedup vs baseline: 1.0209x; 1.0209x over previous
"""CRNN Trainium2 kernel: patchify-conv -> 3x3 conv -> pool -> GRU encoder ->
autoregressive GRU decoder. Pure data-parallel over batch (32 -> 8 cores x 4).

v3: composite conv. conv1 (patchify) and conv2 (3x3) have no nonlinearity
between them, so they fold into 9 tap weights W_t = W2_t @ W1 applied
directly to the fp8 patch buffer (gutter layout, zeros baked host-side).
Tap pairs are M-packed into the 128 PE output partitions with a constant
slot shift between the two halves (delta -8 rows / -1 col), so the whole
conv stack is 15 DoubleRow matmuls + 1 bias matmul per 8-frame group
(was 38). Epilogue combines the shifted PSUM halves with 3 DVE adds.
"""

import os
import sys

for _p in ("/opt/trn_rl_repo", "/root/.axon_site/_ro/trn_rl_repo"):
    if os.path.isdir(_p) and _p not in sys.path:
        sys.path.insert(0, _p)

import numpy as np

import concourse.bass as bass  # noqa: E402
import concourse.mybir as mybir  # noqa: E402
import concourse.tile as tile  # noqa: E402
from concourse import bacc  # noqa: E402
from concourse.bass_utils import run_bass_kernel_spmd  # noqa: E402

F32 = mybir.dt.float32
F8 = mybir.dt.float8e4
AF = mybir.ActivationFunctionType
ALU = mybir.AluOpType
DR = mybir.MatmulPerfMode.DoubleRow

# Model dims (hardcoded from the problem spec)
B, L, DS, DA, DC, DRN, DO, HOR = 32, 16, 12, 16, 64, 256, 2, 10
NCORES, BPC = 8, 4          # batch per core
NG, FPG = 8, 8              # 8 groups of 8 frames per core (frame idx = l*4+b)
BN_EPS = 1e-5
SC = 32.0                   # fp8 composite conv weight scale
LEAD, FW = 16, 16 + 8 * 64 + 16   # 544-col gutter row per patch K-chunk
# composite conv M-pack: P0 pairs (lower off, upper off) share delta -8;
# tap +9 runs solo into P0's lower half; P1 holds the delta -1 pair.
P0_PAIRS = [(-1, -9), (0, -8), (1, -7)]
P1_PAIR = (8, 7)
SOLO_TAP = 9

MM_DT_RNN = os.environ.get("BASS_MM_DT_RNN", "f16")


def _dt_of(tag):
    return {"f32": mybir.dt.float32, "f32r": mybir.dt.float32r,
            "bf16": mybir.dt.bfloat16, "f16": mybir.dt.float16}[tag]

LAST_EXEC_NS = None
LAST_RESULTS = None


def _layout(entries):
    """entries: (name, rows, width[, row0]) -> dict + total cols."""
    out = {}
    cols = 0
    for e in entries:
        name, rows, width = e[0], e[1], e[2]
        row0 = e[3] if len(e) > 3 else 0
        out[name] = (row0, rows, cols, width)
        cols += width
    return out, cols


# matmul operands (RNN matmul dtype)
SMM_LAYOUT, SMM_COLS = _layout([
    ("xt", 12, 64),          # per-core x transposed, col = l*4+b
    ("a0t", 12, 16),
    ("ait", 16, 16),
    ("anT", 80, 256),        # [an_w[:,16:80].T ; an_w[:,0:16].T] rows
    # composite conv bias: rank-10 basis x 0/1 tap-validity patterns
    ("bcT", 10, 64),         # [cnn1_b ; W2_t @ cnn_b per tap] rows
    ("bcP", 10, 512),        # per-slot validity patterns (tiled x8 frames)
    # gate bias-into-psum operands: one matmul per psum tile
    # (lhsT = bias chunks as rows, rhs = chunk->column selector)
    ("brz4", 4, 128),        # (b_ih+b_hh) rz chunks
    ("bni4", 4, 128),        # [b_hh n chunks ; b_ih n chunks] rows
    ("fib2", 2, 128),        # fi_b chunks
    ("sel16", 4, 16),        # selector: col n lights chunk n//4
    ("selni", 4, 16),        # interleaved: even col j -> bhh j//8,
                             # odd col j -> bih j//8
    ("sel8", 2, 8),
])
# bias/affine tables (always fp32)
SMB_LAYOUT, SMB_COLS = _layout([
    ("pscale", 64, 1),       # inv/49/SC
    ("pshift", 64, 1),
    ("a0b", 16, 1),
    ("aib", 16, 1),
    ("anb", 128, 2),         # an_b chunks as cols
    ("fnb", 2, 1),
])

def build_nc():
    nc = bacc.Bacc("TRN2", target_bir_lowering=False, debug=False,
                   num_devices=NCORES)
    mm_rnn = _dt_of(MM_DT_RNN)
    MR = mm_rnn

    # big tensors are DMA'd bitcast to f32: the DMA engines are
    # element-rate limited, so 4x fewer elements = ~4x the bandwidth
    h_fr = nc.dram_tensor("fr", [NG, 128, 6 * FW // 4], F32,
                          kind="ExternalInput")
    h_smm = nc.dram_tensor("smm", [128, SMM_COLS], MR, kind="ExternalInput")
    h_smb = nc.dram_tensor("smb", [128, SMB_COLS], F32, kind="ExternalInput")
    h_wc = nc.dram_tensor("wc", [128, 864], F32, kind="ExternalInput")
    h_wih = nc.dram_tensor("wih", [128, 2 * 768], MR, kind="ExternalInput")
    h_whh = nc.dram_tensor("whh", [128, 2 * 768], MR, kind="ExternalInput")
    h_fi = nc.dram_tensor("fiw", [128, 2 * 256], MR, kind="ExternalInput")
    h_fn = nc.dram_tensor("fnw", [128, 4], MR, kind="ExternalInput")
    h_out = nc.dram_tensor("out", [2, 4 * HOR], F32, kind="ExternalOutput")

    def mm(out, lhsT, rhs, **kw):
        nc.tensor.matmul(out, lhsT, rhs, skip_group_check=True, **kw)

    with tile.TileContext(nc) as tc:
        from contextlib import ExitStack
        with ExitStack() as ctx:
            cpool = ctx.enter_context(tc.tile_pool(name="const", bufs=1))
            work = ctx.enter_context(tc.tile_pool(name="work", bufs=4))
            state = ctx.enter_context(tc.tile_pool(name="state", bufs=1))
            hpool = ctx.enter_context(tc.tile_pool(name="h", bufs=3))
            cps = ctx.enter_context(
                tc.tile_pool(name="cps", bufs=2, space="PSUM"))
            gps = ctx.enter_context(
                tc.tile_pool(name="gps", bufs=2, space="PSUM"))
            psr = ctx.enter_context(
                tc.tile_pool(name="psr", bufs=2, space="PSUM"))

            # ---- constants + frames to SBUF ----
            # Sync queue: g0 frames, conv weights, then the remaining
            # frame groups back-to-back (descriptor issue is ~0.6us each,
            # so one start per group, all resident). Scalar queue: the
            # small RNN weights, done well before first use.
            xins = []
            for g in range(NG):
                xin_g = cpool.tile([128, 6, FW], F8, tag=f"xin{g}")
                xins.append(xin_g)

            def fr_dma(g):
                nc.sync.dma_start(
                    xins[g][:].rearrange("p a b -> p (a b)").bitcast(F32),
                    h_fr[g])

            fr_dma(0)
            wc = cpool.tile([128, 3456], F8, tag="wc")
            nc.sync.dma_start(wc[:].bitcast(F32), h_wc[:])
            for g in range(1, NG):
                fr_dma(g)
            smm = cpool.tile([128, SMM_COLS], MR, tag="smm")
            nc.scalar.dma_start(smm[:], h_smm[:])
            smb = cpool.tile([128, SMB_COLS], F32, tag="smb")
            nc.scalar.dma_start(smb[:], h_smb[:])
            wih = cpool.tile([128, 2 * 768], MR, tag="wih")
            nc.scalar.dma_start(wih[:].bitcast(F32), h_wih[:].bitcast(F32))
            whh = cpool.tile([128, 2 * 768], MR, tag="whh")
            nc.scalar.dma_start(whh[:].bitcast(F32), h_whh[:].bitcast(F32))
            fiw = cpool.tile([128, 2 * 256], MR, tag="fiw")
            nc.scalar.dma_start(fiw[:].bitcast(F32), h_fi[:].bitcast(F32))
            fnw = cpool.tile([128, 4], MR, tag="fnw")
            nc.scalar.dma_start(fnw[:], h_fn[:])

            def sv(name):  # matmul-operand view (RNN dtype)
                r0, rows, off, width = SMM_LAYOUT[name]
                return smm[r0:r0 + rows, off:off + width]

            def svc(name, c0, w):
                r0, rows, off, width = SMM_LAYOUT[name]
                assert c0 + w <= width
                return smm[r0:r0 + rows, off + c0:off + c0 + w]

            def svf(name):  # fp32 bias/affine view
                r0, rows, off, width = SMB_LAYOUT[name]
                return smb[r0:r0 + rows, off:off + width]

            def svcf(name, c0, w):
                r0, rows, off, width = SMB_LAYOUT[name]
                assert c0 + w <= width
                return smb[r0:r0 + rows, off + c0:off + c0 + w]

            # PE warm-up: the tensor engine p-state ramps only under load,
            # and the first real matmuls otherwise run ~2.5x slow. Burn the
            # DMA-wait window (~2.5-10us) with throwaway matmuls on a
            # zeroed tile.
            wtile = work.tile([128, 512], F8, tag="warm")
            nc.vector.memset(wtile[:], 0.0)
            pw = psr.tile([64, 512], F32, tag="ps")
            for wi in range(16):
                mm(pw[:], wtile[:, 0:64], wtile[:, 0:512],
                   start=True, stop=True)

            # ---- persistent state tiles ----
            s2 = state.tile([16, 64], MR, tag="s2")
            s_enc = state.tile([128, 2, 64], MR, tag="senc")
            preds = state.tile([2, 4 * HOR], F32, tag="preds")
            # GRU scan operand tiles (even slots stay zero forever; odd
            # slots rewritten each step). Layout: slot 2i(+1) with pair
            # i = (chunk i//4, batch i%4) matching gate psum column order.
            rzscan = state.tile([128, 32], F32, tag="rzscan")
            nc.vector.memset(rzscan[:], 0.0)
            ozscan = state.tile([128, 16], F32, tag="ozscan")
            nc.gpsimd.memset(ozscan[:], 0.0)
            tzscan = state.tile([128, 16], F32, tag="tzscan")
            nc.gpsimd.memset(tzscan[:], 0.0)
            rz2 = rzscan[:].rearrange("p (a b) -> p a b", b=2)
            oz2 = ozscan[:].rearrange("p (a b) -> p a b", b=2)
            tz2 = tzscan[:].rearrange("p (a b) -> p a b", b=2)

            # ---- state adapters: s1 = relu(a0 x); s2 = s1 + relu(ai s1) ----
            # Emitted mid-group-0 so the conv1 matmuls (which only need
            # w1+xin0) lead the tensor queue instead of stalling on smm.
            def emit_adapters():
                pa = psr.tile([16, 64], F32, tag="ps")
                mm(pa[:], sv("a0t"), sv("xt"), start=True, stop=True)
                s1 = work.tile([16, 64], MR, tag="s1")
                nc.scalar.activation(s1[:], pa[:], AF.Relu, bias=svf("a0b"))
                pb = psr.tile([16, 64], F32, tag="ps")
                mm(pb[:], sv("ait"), s1[:], start=True, stop=True)
                s1b = work.tile([16, 64], MR, tag="s1")
                nc.scalar.activation(s1b[:], pb[:], AF.Relu, bias=svf("aib"))
                nc.vector.tensor_add(s2[:], s1[:], s1b[:])

            # encoder hidden state: odd slots of a scan-layout tile
            h0 = hpool.tile([128, 16], MR, tag="h")
            nc.gpsimd.memset(h0[:], 0.0)
            h_cur = h0[:].rearrange("p (a b) -> p a b", b=2)[:, :, 1]

            def whh_mms(prz16, pni16, hv):
                """whh gate matmuls for one step (the only mms after h)."""
                for mc in range(4):
                    reg = prz16[:, mc * 4:(mc + 1) * 4]
                    for kc in range(2):
                        mm(reg, whh[:, kc * 768 + mc * 128:
                                    kc * 768 + (mc + 1) * 128],
                           hv[:, kc * 4:(kc + 1) * 4],
                           start=False, stop=(mc == 3 and kc == 1))
                pniv = pni16.rearrange("p (c k) -> p c k", c=2)
                for mc2 in range(2):
                    reg = pniv[:, mc2, 0:8:2]
                    for kc in range(2):
                        mm(reg, whh[:, kc * 768 + (4 + mc2) * 128:
                                    kc * 768 + (5 + mc2) * 128],
                           hv[:, kc * 4:(kc + 1) * 4],
                           start=False, stop=(mc2 == 1 and kc == 1))

            def cell_chain(prz16, pni16, hv, after_sigma=None):
                """sigma -> scan(r*ghn+gin) -> tanh -> scan((1-z)n+zh).
                Returns the new hidden as an odd-slot view."""
                nc.scalar.activation(rz2[:, :, 1], prz16, AF.Sigmoid)
                if after_sigma is not None:
                    after_sigma()
                zv = rz2[:, 8:16, 1]
                nc.gpsimd.tensor_scalar(oz2[:, :, 1], zv, -1.0, 1.0,
                                        op0=ALU.mult, op1=ALU.add)
                nc.gpsimd.tensor_mul(tz2[:, :, 1], zv, hv)
                ns = work.tile([128, 16], F32, tag="nscan")
                nc.vector.tensor_tensor_scan(ns[:], rzscan[:, 0:16], pni16,
                                             0.0, op0=ALU.mult, op1=ALU.add)
                nc.scalar.activation(
                    tz2[:, :, 0],
                    ns[:].rearrange("p (a b) -> p a b", b=2)[:, :, 1],
                    AF.Tanh)
                hs = hpool.tile([128, 16], MR, tag="h")
                nc.vector.tensor_tensor_scan(hs[:], ozscan[:], tzscan[:],
                                             0.0, op0=ALU.mult, op1=ALU.add)
                return hs[:].rearrange("p (a b) -> p a b", b=2)[:, :, 1]

            def enc_preload(g):
                """Per-group gate psum tiles for steps 2g, 2g+1: biases +
                wih @ s_enc land before h is even known."""
                # start=True only on the bank's first mm: a start marks the
                # WHOLE psum bank pending-zero, so later first-writes of
                # other regions zero-fill implicitly (start=False).
                ep = gps.tile([128, 5, 16], F32, tag="eg")
                prz, pni = ep[:, 0:2, :], ep[:, 2:4, :]
                for ti in range(2):
                    mm(prz[:, ti, :], sv("brz4"), sv("sel16"),
                       start=(ti == 0), stop=False)
                    mm(pni[:, ti, :], sv("bni4"), sv("selni"),
                       start=False, stop=False)
                # pad write: clears the 16 elements past pni so CoreSim's
                # strided zero-region window never sees mixed state
                mm(ep[:, 4, :], sv("brz4"), sv("sel16"),
                   start=False, stop=True)
                pniv = pni.rearrange("p s (c k) -> p s c k", c=2)
                for ti in range(2):
                    xs = [s_enc[:, kc, g * FPG + ti * 4:g * FPG + ti * 4 + 4]
                          for kc in range(2)]
                    for mc in range(4):
                        reg = prz[:, ti, mc * 4:(mc + 1) * 4]
                        for kc in range(2):
                            mm(reg, wih[:, kc * 768 + mc * 128:
                                        kc * 768 + (mc + 1) * 128],
                               xs[kc], start=False, stop=False)
                    for mc2 in range(2):
                        reg = pniv[:, ti, mc2, 1:8:2]
                        for kc in range(2):
                            mm(reg, wih[:, kc * 768 + (4 + mc2) * 128:
                                        kc * 768 + (5 + mc2) * 128],
                               xs[kc], start=False,
                               stop=(mc2 == 1 and kc == 1))
                return prz, pni

            # ---- composite conv + features, per group of 8 frames ----
            for g in range(NG):
                xin = xins[g]
                pc = cps.tile([128, 2, 512], F32, tag="cps")
                P0, P1 = pc[:, 0, :], pc[:, 1, :]

                def rhs(off, q):
                    a = LEAD + off
                    return xin[:, 2 * q:2 * q + 2, a:a + 512]

                def wcv(blk, m):  # block at col 256*?: [128, 2, m]
                    return wc[:, blk:blk + 2 * m].rearrange(
                        "p (a m) -> p a m", a=2)

                # P0: first pair mm starts the accumulation over all 128
                # rows; bias + solo-tap (M=64) mms ride in the middle; the
                # last pair mm closes the group.
                for p, (lo, up) in enumerate(P0_PAIRS):
                    for q in range(3):
                        first = (p == 0 and q == 0)
                        last = (p == 2 and q == 2)
                        mm(P0, wcv((p * 3 + q) * 256, 128), rhs(lo, q),
                           start=first, stop=last, perf_mode=DR)
                        if first:
                            # conv bias (rank-10 basis x validity patterns)
                            mm(P0[0:64], sv("bcT"), sv("bcP"),
                               start=False, stop=False)
                            for q2 in range(3):
                                mm(P0[0:64], wcv(2304 + q2 * 128, 64),
                                   rhs(SOLO_TAP, q2),
                                   start=False, stop=False, perf_mode=DR)
                for q in range(3):
                    mm(P1, wcv(2688 + q * 256, 128), rhs(P1_PAIR[0], q),
                       start=(q == 0), stop=(q == 2), perf_mode=DR)

                if g == 0:
                    emit_adapters()

                # epilogue: combine shifted psum halves -> relu -> sum ->
                # affine. upper halves hold the paired tap accumulated at
                # slot+delta (P0 delta -8 = one grid row, P1 delta -1).
                p0g = pc[0:64, 0, :].rearrange("p (f a b) -> p f a b",
                                               a=8, b=8)
                p0u = pc[64:128, 0, :].rearrange("p (f a b) -> p f a b",
                                                 a=8, b=8)
                p1g = pc[0:64, 1, :].rearrange("p (f a b) -> p f a b",
                                               a=8, b=8)
                p1u = pc[64:128, 1, :].rearrange("p (f a b) -> p f a b",
                                                 a=8, b=8)
                # DVE/ACT ops cannot read two PSUM operands in one
                # instruction: stage the upper halves through SBUF.
                u0 = work.tile([64, 8, 7, 7], F32, tag="epu0")
                nc.scalar.activation(u0[:], p0u[:, :, 0:7, 1:8], AF.Copy)
                u1 = work.tile([64, 8, 7, 7], F32, tag="epu1")
                nc.vector.tensor_copy(u1[:], p1u[:, :, 1:8, 0:7])
                t0 = work.tile([64, 8, 7, 7], F32, tag="ep0")
                nc.vector.tensor_add(t0[:], p0g[:, :, 1:8, 1:8], u0[:])
                t1 = work.tile([64, 8, 7, 7], F32, tag="ep1")
                nc.vector.tensor_add(t1[:], p1g[:, :, 1:8, 1:8], u1[:])
                t2 = work.tile([64, 8, 7, 7], F32, tag="ep2")
                nc.vector.tensor_add(t2[:], t0[:], t1[:])
                t3 = work.tile([64, 8, 7, 7], F32, tag="ep")
                nc.vector.tensor_scalar_max(t3[:], t2[:], 0.0)
                red = work.tile([64, 8], F32, tag="red")
                nc.vector.tensor_reduce(red[:], t3[:],
                                        axis=mybir.AxisListType.XY,
                                        op=ALU.add)
                feats = work.tile([80, 8], MR, tag="feats")
                nc.scalar.activation(feats[0:64, :], red[:], AF.Identity,
                                     bias=svf("pshift"), scale=svf("pscale"))

                # an: relu(an_w [s2; feats] + an_b), one K=80 matmul per half
                gcol = slice(g * FPG, (g + 1) * FPG)
                nc.gpsimd.tensor_copy(feats[64:80, :], s2[:, gcol])
                for mc in range(2):
                    pan = psr.tile([128, FPG], F32, tag="ps")
                    mm(pan[:], svc("anT", mc * 128, 128), feats[:],
                       start=True, stop=True)
                    nc.scalar.activation(s_enc[:, mc, gcol], pan[:], AF.Relu,
                                         bias=svcf("anb", mc, 1))

                # encoder steps that become ready after this group
                eprz, epni = enc_preload(g)
                for ti in range(2):
                    whh_mms(eprz[:, ti, :], epni[:, ti, :], h_cur)
                    h_cur = cell_chain(eprz[:, ti, :], epni[:, ti, :], h_cur)

            # ---- decoder ----
            # Emission order puts everything that depends only on hn(t-1)
            # (whh parts) ahead of the xr(t-1)-dependent wih work, and
            # defers fn(t-1) behind the whh block, so the tensor queue
            # keeps moving during the fi/xr window.
            def emit_fn(x, tt):
                pfn = psr.tile([2, 4], F32, tag="ps")
                for kc in range(2):
                    mm(pfn[:], fnw[:, kc * 2:(kc + 1) * 2],
                       x[:, kc * 4:(kc + 1) * 4],
                       start=(kc == 0), stop=(kc == 1))
                nc.scalar.activation(preds[:, tt * 4:(tt + 1) * 4], pfn[:],
                                     AF.Tanh, bias=svf("fnb"))

            xi, hh = h_cur, h_cur
            for t in range(HOR):
                dp = gps.tile([128, 5, 16], F32, tag="eg")
                prz, pni = dp[:, 0, :], dp[:, 1, :]
                mm(prz, sv("brz4"), sv("sel16"), start=True, stop=False)
                mm(pni, sv("bni4"), sv("selni"), start=False, stop=False)
                pniv = pni.rearrange("p (c k) -> p c k", c=2)
                # whh parts (ready at hh)
                for mc in range(4):
                    reg = prz[:, mc * 4:(mc + 1) * 4]
                    for kc in range(2):
                        mm(reg, whh[:, kc * 768 + mc * 128:
                                    kc * 768 + (mc + 1) * 128],
                           hh[:, kc * 4:(kc + 1) * 4],
                           start=False, stop=False)
                for mc2 in range(2):
                    reg = pniv[:, mc2, 0:8:2]
                    for kc in range(2):
                        mm(reg, whh[:, kc * 768 + (4 + mc2) * 128:
                                    kc * 768 + (5 + mc2) * 128],
                           hh[:, kc * 4:(kc + 1) * 4],
                           start=False, stop=False)
                if t > 0:
                    emit_fn(xi, t - 1)
                # wih parts (ready at xi)
                mm(dp[:, 2, :], sv("brz4"), sv("sel16"),
                   start=False, stop=True)
                for mc in range(4):
                    reg = prz[:, mc * 4:(mc + 1) * 4]
                    for kc in range(2):
                        mm(reg, wih[:, kc * 768 + mc * 128:
                                    kc * 768 + (mc + 1) * 128],
                           xi[:, kc * 4:(kc + 1) * 4],
                           start=False, stop=(mc == 3 and kc == 1))
                for mc2 in range(2):
                    reg = pniv[:, mc2, 1:8:2]
                    for kc in range(2):
                        mm(reg, wih[:, kc * 768 + (4 + mc2) * 128:
                                    kc * 768 + (5 + mc2) * 128],
                           xi[:, kc * 4:(kc + 1) * 4],
                           start=False, stop=(mc2 == 1 and kc == 1))
                # fi bias rides the idle tensor window before hn is ready
                pfi = psr.tile([128, 8], F32, tag="ps")
                mm(pfi[:], sv("fib2"), sv("sel8"), start=True, stop=False)

                hn = cell_chain(prz, pni, hh)

                # final_i residual: xr = hn + relu(fi hn + fi_b)
                for mc2 in range(2):
                    reg = pfi[:, mc2 * 4:(mc2 + 1) * 4]
                    for kc2 in range(2):
                        mm(reg, fiw[:, kc2 * 256 + mc2 * 128:
                                    kc2 * 256 + (mc2 + 1) * 128],
                           hn[:, kc2 * 4:(kc2 + 1) * 4],
                           start=False, stop=(mc2 == 1 and kc2 == 1))
                xr = hpool.tile([128, 8], MR, tag="xr")
                nc.vector.scalar_tensor_tensor(
                    xr[:], pfi[:], 0.0, hn,
                    op0=ALU.max, op1=ALU.add)
                xi, hh = xr[:], hn
            emit_fn(xi, HOR - 1)

            nc.sync.dma_start(h_out[:], preds[:])

    nc.finalize()
    return nc


# ---------------- host-side data prep ----------------

def _prep_frames(frames):
    """frames (32,16,3,112,112) -> per-core [NG, 128, 6*FW] gutter-layout
    patch-T fp8 (8x8 cell grid per frame, row0/col0 + LEAD/TAIL zeros)."""
    out = np.empty((NCORES, NG, 128, 6 * FW), mybir.dt.np(F8))
    fr = np.ascontiguousarray(frames, np.float32)
    for c in range(NCORES):
        fb = fr[c * BPC:(c + 1) * BPC]  # (4, 16, 3, 112, 112)
        a = fb.reshape(BPC, L, 3, 7, 16, 7, 16)
        # -> [l, b, ch, kh, kw, ph, pw]
        a = a.transpose(1, 0, 2, 4, 6, 3, 5)
        a = a.reshape(L, BPC, 768, 49)
        a = a.reshape(NG, 2, BPC, 6, 128, 49)
        # -> [g, k, p, li, b, s]
        a = a.transpose(0, 3, 4, 1, 2, 5)
        a = a.reshape(NG, 6, 128, 8, 7, 7)
        buf = np.zeros((NG, 6, 128, FW), np.float32)
        grid = buf[:, :, :, LEAD:LEAD + 512].reshape(NG, 6, 128, 8, 8, 8)
        grid[:, :, :, :, 1:8, 1:8] = a
        out[c] = buf.transpose(0, 2, 1, 3).reshape(
            NG, 128, 6 * FW).astype(mybir.dt.np(F8))
    return out


def _tap_weights(iv):
    """Composite per-tap weights W_t = cnn1_w[:,:,dh,dw] @ W1 (64, 768),
    keyed by gutter-slot offset (dh-1)*8 + (dw-1), scaled by SC."""
    W1f = iv["cnn_w"].reshape(576, 768).astype(np.float32)
    T = {}
    for dh in range(3):
        for dw in range(3):
            off = (dh - 1) * 8 + (dw - 1)
            T[off] = (iv["cnn1_w"][:, :, dh, dw].astype(np.float32)
                      @ W1f) * SC
    return T


def _prep_weights(iv):
    w = {}
    f8 = mybir.dt.np(F8)
    T = _tap_weights(iv)

    # composite conv lhsT blocks, in matmul emission order:
    # 9x [128, 2, 128] P0 pair blocks, 3x [128, 2, 64] solo-tap blocks,
    # 3x [128, 2, 128] P1 pair blocks.
    wcb = np.zeros((128, 3456), np.float32)
    col = 0
    for pair in P0_PAIRS + [None, P1_PAIR]:
        for q in range(3):
            if pair is None:
                blk = np.zeros((128, 2, 64), np.float32)
                for j in range(2):
                    c = 2 * q + j
                    blk[:, j, :] = T[SOLO_TAP][:, c * 128:(c + 1) * 128].T
                wcb[:, col:col + 128] = blk.reshape(128, 128)
                col += 128
            else:
                lo, up = pair
                blk = np.zeros((128, 2, 128), np.float32)
                for j in range(2):
                    c = 2 * q + j
                    blk[:, j, 0:64] = T[lo][:, c * 128:(c + 1) * 128].T
                    blk[:, j, 64:128] = T[up][:, c * 128:(c + 1) * 128].T
                wcb[:, col:col + 256] = blk.reshape(128, 256)
                col += 256
    assert col == 3456
    w["wc"] = np.ascontiguousarray(wcb).astype(f8).view(np.float32)

    rdt = mybir.dt.np(_dt_of(MM_DT_RNN))
    for name, key in (("wih", "w_ih"), ("whh", "w_hh")):
        T = iv[key].T.astype(np.float32)  # (256, 768)
        w[name] = np.ascontiguousarray(
            T.reshape(2, 128, 768).transpose(1, 0, 2).reshape(
                128, 1536)).astype(rdt)
    T = iv["fi_w"].T.astype(np.float32)  # (256, 256)
    w["fiw"] = np.ascontiguousarray(
        T.reshape(2, 128, 256).transpose(1, 0, 2).reshape(128, 512)).astype(rdt)
    T = iv["fn_w"].T.astype(np.float32)  # (256, 2)
    w["fnw"] = np.ascontiguousarray(
        T.reshape(2, 128, 2).transpose(1, 0, 2).reshape(128, 4)).astype(rdt)
    return w


def _prep_smalls(iv, x, core):
    smm = np.zeros((128, SMM_COLS), mybir.dt.np(_dt_of(MM_DT_RNN)))
    smb = np.zeros((128, SMB_COLS), np.float32)

    def put(name, arr):
        if name in SMM_LAYOUT:
            r0, rows, off, width = SMM_LAYOUT[name]
            dst = smm
        else:
            r0, rows, off, width = SMB_LAYOUT[name]
            dst = smb
        a = np.asarray(arr, np.float32).reshape(rows, width)
        dst[r0:r0 + rows, off:off + width] = a.astype(dst.dtype)

    # composite conv bias = cnn1_b + sum over in-range taps of
    # (W2_t @ cnn_b): rank-10 basis (bcT) x 0/1 validity patterns (bcP),
    # folded into PSUM by one matmul; x SC to match the psum scale
    M = np.einsum("oiab,i->oab", iv["cnn1_w"], iv["cnn_b"]).astype(np.float32)
    bct = np.zeros((10, 64), np.float32)
    bcp = np.zeros((10, 512), np.float32)
    bct[0] = iv["cnn1_b"].astype(np.float32)
    bcp[0] = 1.0
    grid = bcp.reshape(10, 8, 8, 8)
    ti = 1
    for dh in range(3):
        for dw in range(3):
            bct[ti] = M[:, dh, dw]
            for r in range(8):
                for cc in range(8):
                    if 2 <= r + dh <= 8 and 2 <= cc + dw <= 8:
                        grid[ti, :, r, cc] = 1.0
            ti += 1
    put("bcT", bct * SC)
    put("bcP", bcp)

    inv = iv["bn_g"] / np.sqrt(iv["bn_v"] + BN_EPS)
    put("pscale", (inv / 49.0 / SC)[:, None])
    put("pshift", (iv["bn_b"] - iv["bn_m"] * inv)[:, None])

    xb = x[core * BPC:(core + 1) * BPC]  # (4, 16, 12)
    put("xt", xb.transpose(2, 1, 0).reshape(12, 64))

    put("a0t", iv["a0_w"].T)
    put("a0b", iv["a0_b"][:, None])
    put("ait", iv["ai_w"].T)
    put("aib", iv["ai_b"][:, None])
    put("anT", np.concatenate([iv["an_w"][:, 16:80].T,
                               iv["an_w"][:, 0:16].T], axis=0))
    put("anb", iv["an_b"].reshape(2, 128).T)

    put("fnb", iv["fn_b"][:, None])
    bs = (iv["b_ih"] + iv["b_hh"]).astype(np.float32)
    put("brz4", bs[:512].reshape(4, 128))
    # n-gate interleaved bias: rows [bhh c0, bhh c1, bih c0, bih c1];
    # slot j = c*8 + b*2 + parity -> row parity*2 + c
    put("bni4", np.concatenate([iv["b_hh"][512:].reshape(2, 128),
                                iv["b_ih"][512:].reshape(2, 128)]))
    selni = np.zeros((4, 16), np.float32)
    for j in range(16):
        selni[(j % 2) * 2 + j // 8, j] = 1.0
    put("selni", selni)
    put("fib2", iv["fi_b"].reshape(2, 128))
    put("sel16", np.repeat(np.eye(4, dtype=np.float32), 4, axis=1))
    put("sel8", np.repeat(np.eye(2, dtype=np.float32), 4, axis=1))
    return smm, smb


def make_in_maps(inputs):
    iv = {k: np.asarray(v, np.float32) for k, v in inputs.items()}
    frames = iv["frames"]
    x = iv["x"]
    fr_all = _prep_frames(frames)
    w = _prep_weights(iv)
    in_maps = []
    for c in range(NCORES):
        smm, smb = _prep_smalls(iv, x, c)
        m = {"fr": np.ascontiguousarray(fr_all[c]).view(np.float32),
             "smm": smm, "smb": smb}
        m.update(w)
        in_maps.append(m)
    return in_maps


_NC_CACHE = None


def get_nc():
    global _NC_CACHE
    if _NC_CACHE is None:
        _NC_CACHE = build_nc()
    return _NC_CACHE


def _install_ntff_hook():
    """The agent image's antenv lacks axon_hooks; synthesize it so
    run_bass_kernel_spmd(trace=True) can capture NTFF profiles."""
    try:
        from antenv.axon_hooks import get_axon_ntff_profile_hook  # noqa: F401
        return True
    except ImportError:
        pass
    try:
        import types
        import antenv
        if "/root/.axon_site" not in sys.path:
            sys.path.insert(0, "/root/.axon_site")
        from trn_agent_boot.trn_boot import _ntff_profile_via_ctypes
        hook = _ntff_profile_via_ctypes("/opt/axon/libaxon_pjrt.so")
        mod = types.ModuleType("antenv.axon_hooks")
        mod.get_axon_ntff_profile_hook = lambda: hook
        mod.set_axon_ntff_profile_hook = lambda h: None
        sys.modules["antenv.axon_hooks"] = mod
        antenv.axon_hooks = mod
        return hook is not None
    except Exception as e:  # pragma: no cover - profiling is best-effort
        print(f"ntff hook install failed: {e}")
        return False


def kernel(**inputs):
    global LAST_EXEC_NS, LAST_RESULTS
    nc = get_nc()
    in_maps = make_in_maps(inputs)
    trace = bool(int(os.environ.get("KERNEL_TRACE", "0")))
    if trace:
        trace = _install_ntff_hook()
    res = run_bass_kernel_spmd(nc, in_maps, core_ids=list(range(NCORES)),
                               trace=trace)
    LAST_RESULTS = res
    LAST_EXEC_NS = res.exec_time_ns
    outs = []
    for c in range(NCORES):
        o = res.results[c]["out"]  # (2, 40)
        outs.append(o.reshape(2, HOR, BPC).transpose(1, 2, 0)[:, :, None, :])
    return np.concatenate(outs, axis=1).astype(np.float32)


if __name__ == "__main__":
    nc = get_nc()
    print("built ok; instructions:",
          sum(len(bb.instructions) for bb in nc.main_func.blocks))



# revision 46
# speedup vs baseline: 1.0529x; 1.0314x over previous
"""CRNN Trainium2 kernel: patchify-conv -> 3x3 conv -> pool -> GRU encoder ->
autoregressive GRU decoder. Pure data-parallel over batch (32 -> 8 cores x 4).

v3: composite conv. conv1 (patchify) and conv2 (3x3) have no nonlinearity
between them, so they fold into 9 tap weights W_t = W2_t @ W1 applied
directly to the fp8 patch buffer (gutter layout, zeros baked host-side).
Tap pairs are M-packed into the 128 PE output partitions with a constant
slot shift between the two halves (delta -8 rows / -1 col), so the whole
conv stack is 15 DoubleRow matmuls + 1 bias matmul per 8-frame group
(was 38). Epilogue combines the shifted PSUM halves with 3 DVE adds.
"""

import os
import sys

for _p in ("/opt/trn_rl_repo", "/root/.axon_site/_ro/trn_rl_repo"):
    if os.path.isdir(_p) and _p not in sys.path:
        sys.path.insert(0, _p)

import numpy as np

import concourse.bass as bass  # noqa: E402
import concourse.mybir as mybir  # noqa: E402
import concourse.tile as tile  # noqa: E402
from concourse import bacc  # noqa: E402
from concourse.bass_utils import run_bass_kernel_spmd  # noqa: E402

F32 = mybir.dt.float32
F8 = mybir.dt.float8e4
AF = mybir.ActivationFunctionType
ALU = mybir.AluOpType
DR = mybir.MatmulPerfMode.DoubleRow

# Model dims (hardcoded from the problem spec)
B, L, DS, DA, DC, DRN, DO, HOR = 32, 16, 12, 16, 64, 256, 2, 10
NCORES, BPC = 8, 4          # batch per core
NG, FPG = 8, 8              # 8 groups of 8 frames per core (frame idx = l*4+b)
BN_EPS = 1e-5
SC = 32.0                   # fp8 composite conv weight scale
LEAD, FW = 16, 16 + 8 * 64 + 16   # 544-col gutter row per patch K-chunk
# composite conv M-pack: P0 pairs (lower off, upper off) share delta -8;
# tap +9 runs solo into P0's lower half; P1 holds the delta -1 pair.
P0_PAIRS = [(-1, -9), (0, -8), (1, -7)]
P1_PAIR = (8, 7)
SOLO_TAP = 9

MM_DT_RNN = os.environ.get("BASS_MM_DT_RNN", "f16")


def _dt_of(tag):
    return {"f32": mybir.dt.float32, "f32r": mybir.dt.float32r,
            "bf16": mybir.dt.bfloat16, "f16": mybir.dt.float16}[tag]

LAST_EXEC_NS = None
LAST_RESULTS = None


def _layout(entries):
    """entries: (name, rows, width[, row0]) -> dict + total cols."""
    out = {}
    cols = 0
    for e in entries:
        name, rows, width = e[0], e[1], e[2]
        row0 = e[3] if len(e) > 3 else 0
        out[name] = (row0, rows, cols, width)
        cols += width
    return out, cols


# matmul operands (RNN matmul dtype)
SMM_LAYOUT, SMM_COLS = _layout([
    ("xt", 12, 64),          # per-core x transposed, col = l*4+b
    ("a0t", 12, 16),
    ("ait", 16, 16),
    ("anT", 80, 256),        # [an_w[:,16:80].T ; an_w[:,0:16].T] rows
    # composite conv bias: rank-10 basis x 0/1 tap-validity patterns
    ("bcT", 10, 64),         # [cnn1_b ; W2_t @ cnn_b per tap] rows
    ("bcP", 10, 512),        # per-slot validity patterns (tiled x8 frames)
    # gate bias-into-psum operands: one matmul per psum tile
    # (lhsT = bias chunks as rows, rhs = chunk->column selector)
    ("brz4", 4, 128),        # (b_ih+b_hh) rz chunks
    ("bni4", 4, 128),        # [b_hh n chunks ; b_ih n chunks] rows
    ("fib2", 2, 128),        # fi_b chunks
    ("sel16", 4, 16),        # selector: col n lights chunk n//4
    ("selni", 4, 16),        # interleaved: even col j -> bhh j//8,
                             # odd col j -> bih j//8
    ("sel8", 2, 8),
])
# bias/affine tables (always fp32)
SMB_LAYOUT, SMB_COLS = _layout([
    ("pscale", 64, 1),       # inv/49/SC
    ("pshift", 64, 1),
    ("a0b", 16, 1),
    ("aib", 16, 1),
    ("anb", 128, 2),         # an_b chunks as cols
    ("fnb", 2, 1),
])

def build_nc():
    nc = bacc.Bacc("TRN2", target_bir_lowering=False, debug=False,
                   num_devices=NCORES)
    mm_rnn = _dt_of(MM_DT_RNN)
    MR = mm_rnn

    # big tensors are DMA'd bitcast to f32: the DMA engines are
    # element-rate limited, so 4x fewer elements = ~4x the bandwidth
    h_fr = nc.dram_tensor("fr", [NG, 128, 6 * FW // 4], F32,
                          kind="ExternalInput")
    h_smm = nc.dram_tensor("smm", [128, SMM_COLS], MR, kind="ExternalInput")
    h_smb = nc.dram_tensor("smb", [128, SMB_COLS], F32, kind="ExternalInput")
    h_wc = nc.dram_tensor("wc", [128, 864], F32, kind="ExternalInput")
    h_wih = nc.dram_tensor("wih", [128, 2 * 768], MR, kind="ExternalInput")
    h_whh = nc.dram_tensor("whh", [128, 2 * 768], MR, kind="ExternalInput")
    h_fi = nc.dram_tensor("fiw", [128, 2 * 256], MR, kind="ExternalInput")
    h_fn = nc.dram_tensor("fnw", [128, 4], MR, kind="ExternalInput")
    h_out = nc.dram_tensor("out", [2, 4 * HOR], F32, kind="ExternalOutput")

    def mm(out, lhsT, rhs, **kw):
        nc.tensor.matmul(out, lhsT, rhs, skip_group_check=True, **kw)

    with tile.TileContext(nc) as tc:
        from contextlib import ExitStack
        with ExitStack() as ctx:
            cpool = ctx.enter_context(tc.tile_pool(name="const", bufs=1))
            work = ctx.enter_context(tc.tile_pool(name="work", bufs=4))
            state = ctx.enter_context(tc.tile_pool(name="state", bufs=1))
            hpool = ctx.enter_context(tc.tile_pool(name="h", bufs=3))
            cps = ctx.enter_context(
                tc.tile_pool(name="cps", bufs=2, space="PSUM"))
            gps = ctx.enter_context(
                tc.tile_pool(name="gps", bufs=2, space="PSUM"))
            psr = ctx.enter_context(
                tc.tile_pool(name="psr", bufs=2, space="PSUM"))

            # ---- constants + frames to SBUF ----
            # Sync queue: g0 frames, conv weights, then the remaining
            # frame groups back-to-back (descriptor issue is ~0.6us each,
            # so one start per group, all resident). Scalar queue: the
            # small RNN weights, done well before first use.
            xins = []
            for g in range(NG):
                xin_g = cpool.tile([128, 6, FW], F8, tag=f"xin{g}")
                xins.append(xin_g)

            def fr_dma(g):
                nc.sync.dma_start(
                    xins[g][:].rearrange("p a b -> p (a b)").bitcast(F32),
                    h_fr[g])

            fr_dma(0)
            wc = cpool.tile([128, 3456], F8, tag="wc")
            nc.sync.dma_start(wc[:].bitcast(F32), h_wc[:])
            for g in range(1, NG):
                fr_dma(g)
            smm = cpool.tile([128, SMM_COLS], MR, tag="smm")
            nc.scalar.dma_start(smm[:], h_smm[:])
            smb = cpool.tile([128, SMB_COLS], F32, tag="smb")
            nc.scalar.dma_start(smb[:], h_smb[:])
            wih = cpool.tile([128, 2 * 768], MR, tag="wih")
            nc.scalar.dma_start(wih[:].bitcast(F32), h_wih[:].bitcast(F32))
            whh = cpool.tile([128, 2 * 768], MR, tag="whh")
            nc.scalar.dma_start(whh[:].bitcast(F32), h_whh[:].bitcast(F32))
            fiw = cpool.tile([128, 2 * 256], MR, tag="fiw")
            nc.scalar.dma_start(fiw[:].bitcast(F32), h_fi[:].bitcast(F32))
            fnw = cpool.tile([128, 4], MR, tag="fnw")
            nc.scalar.dma_start(fnw[:], h_fn[:])

            def sv(name):  # matmul-operand view (RNN dtype)
                r0, rows, off, width = SMM_LAYOUT[name]
                return smm[r0:r0 + rows, off:off + width]

            def svc(name, c0, w):
                r0, rows, off, width = SMM_LAYOUT[name]
                assert c0 + w <= width
                return smm[r0:r0 + rows, off + c0:off + c0 + w]

            def svf(name):  # fp32 bias/affine view
                r0, rows, off, width = SMB_LAYOUT[name]
                return smb[r0:r0 + rows, off:off + width]

            def svcf(name, c0, w):
                r0, rows, off, width = SMB_LAYOUT[name]
                assert c0 + w <= width
                return smb[r0:r0 + rows, off + c0:off + c0 + w]

            # PE warm-up: the tensor engine p-state ramps only under load,
            # and the first real matmuls otherwise run ~2.5x slow. Burn the
            # DMA-wait window (~2.5-10us) with throwaway matmuls on a
            # zeroed tile.
            wtile = work.tile([128, 512], F8, tag="warm")
            nc.vector.memset(wtile[:], 0.0)
            pw = psr.tile([64, 512], F32, tag="ps")
            for wi in range(16):
                mm(pw[:], wtile[:, 0:64], wtile[:, 0:512],
                   start=True, stop=True)

            # ---- persistent state tiles ----
            s2 = state.tile([16, 64], MR, tag="s2")
            s_enc = state.tile([128, 2, 64], MR, tag="senc")
            preds = state.tile([2, 4 * HOR], F32, tag="preds")
            # GRU scan operand tiles (even slots stay zero forever; odd
            # slots rewritten each step). Layout: slot 2i(+1) with pair
            # i = (chunk i//4, batch i%4) matching gate psum column order.
            rzscan = state.tile([128, 32], F32, tag="rzscan")
            nc.vector.memset(rzscan[:], 0.0)
            ozscan = state.tile([128, 16], F32, tag="ozscan")
            nc.gpsimd.memset(ozscan[:], 0.0)
            tzscan = state.tile([128, 16], F32, tag="tzscan")
            nc.gpsimd.memset(tzscan[:], 0.0)
            rz2 = rzscan[:].rearrange("p (a b) -> p a b", b=2)
            oz2 = ozscan[:].rearrange("p (a b) -> p a b", b=2)
            tz2 = tzscan[:].rearrange("p (a b) -> p a b", b=2)

            # ---- state adapters: s1 = relu(a0 x); s2 = s1 + relu(ai s1) ----
            # Emitted mid-group-0 so the conv1 matmuls (which only need
            # w1+xin0) lead the tensor queue instead of stalling on smm.
            def emit_adapters():
                pa = psr.tile([16, 64], F32, tag="ps")
                mm(pa[:], sv("a0t"), sv("xt"), start=True, stop=True)
                s1 = work.tile([16, 64], MR, tag="s1")
                nc.scalar.activation(s1[:], pa[:], AF.Relu, bias=svf("a0b"))
                pb = psr.tile([16, 64], F32, tag="ps")
                mm(pb[:], sv("ait"), s1[:], start=True, stop=True)
                s1b = work.tile([16, 64], MR, tag="s1")
                nc.scalar.activation(s1b[:], pb[:], AF.Relu, bias=svf("aib"))
                nc.vector.tensor_add(s2[:], s1[:], s1b[:])

            # encoder hidden state: odd slots of a scan-layout tile
            h0 = hpool.tile([128, 16], MR, tag="h")
            nc.gpsimd.memset(h0[:], 0.0)
            h_cur = h0[:].rearrange("p (a b) -> p a b", b=2)[:, :, 1]

            def whh_mms(prz16, pni16, hv):
                """whh gate matmuls for one step (the only mms after h)."""
                for mc in range(4):
                    reg = prz16[:, mc * 4:(mc + 1) * 4]
                    for kc in range(2):
                        mm(reg, whh[:, kc * 768 + mc * 128:
                                    kc * 768 + (mc + 1) * 128],
                           hv[:, kc * 4:(kc + 1) * 4],
                           start=False, stop=(mc == 3 and kc == 1))
                pniv = pni16.rearrange("p (c k) -> p c k", c=2)
                for mc2 in range(2):
                    reg = pniv[:, mc2, 0:8:2]
                    for kc in range(2):
                        mm(reg, whh[:, kc * 768 + (4 + mc2) * 128:
                                    kc * 768 + (5 + mc2) * 128],
                           hv[:, kc * 4:(kc + 1) * 4],
                           start=False, stop=(mc2 == 1 and kc == 1))

            def cell_chain(prz16, pni16, hv, after_sigma=None):
                """sigma -> scan(r*ghn+gin) -> tanh -> scan((1-z)n+zh).
                Returns the new hidden as an odd-slot view."""
                nc.scalar.activation(rz2[:, :, 1], prz16, AF.Sigmoid)
                if after_sigma is not None:
                    after_sigma()
                zv = rz2[:, 8:16, 1]
                nc.gpsimd.tensor_scalar(oz2[:, :, 1], zv, -1.0, 1.0,
                                        op0=ALU.mult, op1=ALU.add)
                nc.gpsimd.tensor_mul(tz2[:, :, 1], zv, hv)
                ns = work.tile([128, 16], F32, tag="nscan")
                nc.vector.tensor_tensor_scan(ns[:], rzscan[:, 0:16], pni16,
                                             0.0, op0=ALU.mult, op1=ALU.add)
                nc.scalar.activation(
                    tz2[:, :, 0],
                    ns[:].rearrange("p (a b) -> p a b", b=2)[:, :, 1],
                    AF.Tanh)
                hs = hpool.tile([128, 16], MR, tag="h")
                nc.vector.tensor_tensor_scan(hs[:], ozscan[:], tzscan[:],
                                             0.0, op0=ALU.mult, op1=ALU.add)
                return hs[:].rearrange("p (a b) -> p a b", b=2)[:, :, 1]

            def enc_preload(g):
                """Per-group gate psum tiles for steps 2g, 2g+1: biases +
                wih @ s_enc land before h is even known."""
                # start=True only on the bank's first mm: a start marks the
                # WHOLE psum bank pending-zero, so later first-writes of
                # other regions zero-fill implicitly (start=False).
                ep = gps.tile([128, 5, 16], F32, tag="eg")
                prz, pni = ep[:, 0:2, :], ep[:, 2:4, :]
                for ti in range(2):
                    mm(prz[:, ti, :], sv("brz4"), sv("sel16"),
                       start=(ti == 0), stop=False)
                    mm(pni[:, ti, :], sv("bni4"), sv("selni"),
                       start=False, stop=False)
                # pad write: clears the 16 elements past pni so CoreSim's
                # strided zero-region window never sees mixed state
                mm(ep[:, 4, :], sv("brz4"), sv("sel16"),
                   start=False, stop=True)
                pniv = pni.rearrange("p s (c k) -> p s c k", c=2)
                for ti in range(2):
                    xs = [s_enc[:, kc, g * FPG + ti * 4:g * FPG + ti * 4 + 4]
                          for kc in range(2)]
                    for mc in range(4):
                        reg = prz[:, ti, mc * 4:(mc + 1) * 4]
                        for kc in range(2):
                            mm(reg, wih[:, kc * 768 + mc * 128:
                                        kc * 768 + (mc + 1) * 128],
                               xs[kc], start=False, stop=False)
                    for mc2 in range(2):
                        reg = pniv[:, ti, mc2, 1:8:2]
                        for kc in range(2):
                            mm(reg, wih[:, kc * 768 + (4 + mc2) * 128:
                                        kc * 768 + (5 + mc2) * 128],
                               xs[kc], start=False,
                               stop=(mc2 == 1 and kc == 1))
                return prz, pni

            # ---- composite conv + features, per group of 8 frames ----
            for g in range(NG):
                xin = xins[g]
                pc = cps.tile([128, 2, 512], F32, tag="cps")
                P0, P1 = pc[:, 0, :], pc[:, 1, :]

                def rhs(off, q):
                    a = LEAD + off
                    return xin[:, 2 * q:2 * q + 2, a:a + 512]

                def wcv(blk, m):  # block at col 256*?: [128, 2, m]
                    return wc[:, blk:blk + 2 * m].rearrange(
                        "p (a m) -> p a m", a=2)

                # P0: first pair mm starts the accumulation over all 128
                # rows; bias + solo-tap (M=64) mms ride in the middle; the
                # last pair mm closes the group.
                for p, (lo, up) in enumerate(P0_PAIRS):
                    for q in range(3):
                        first = (p == 0 and q == 0)
                        last = (p == 2 and q == 2)
                        mm(P0, wcv((p * 3 + q) * 256, 128), rhs(lo, q),
                           start=first, stop=last, perf_mode=DR)
                        if first:
                            # conv bias (rank-10 basis x validity patterns)
                            mm(P0[0:64], sv("bcT"), sv("bcP"),
                               start=False, stop=False)
                            for q2 in range(3):
                                mm(P0[0:64], wcv(2304 + q2 * 128, 64),
                                   rhs(SOLO_TAP, q2),
                                   start=False, stop=False, perf_mode=DR)
                for q in range(3):
                    mm(P1, wcv(2688 + q * 256, 128), rhs(P1_PAIR[0], q),
                       start=(q == 0), stop=(q == 2), perf_mode=DR)

                if g == 0:
                    emit_adapters()

                # epilogue: combine shifted psum halves -> relu -> sum ->
                # affine. upper halves hold the paired tap accumulated at
                # slot+delta (P0 delta -8 = one grid row, P1 delta -1).
                p0g = pc[0:64, 0, :].rearrange("p (f a b) -> p f a b",
                                               a=8, b=8)
                p0u = pc[64:128, 0, :].rearrange("p (f a b) -> p f a b",
                                                 a=8, b=8)
                p1g = pc[0:64, 1, :].rearrange("p (f a b) -> p f a b",
                                               a=8, b=8)
                p1u = pc[64:128, 1, :].rearrange("p (f a b) -> p f a b",
                                                 a=8, b=8)
                # DVE/ACT ops cannot read two PSUM operands in one
                # instruction: stage the upper halves through SBUF.
                u0 = work.tile([64, 8, 7, 7], F32, tag="epu0")
                nc.scalar.activation(u0[:], p0u[:, :, 0:7, 1:8], AF.Copy)
                u1 = work.tile([64, 8, 7, 7], F32, tag="epu1")
                nc.scalar.activation(u1[:], p1u[:, :, 1:8, 0:7], AF.Copy)
                t0 = work.tile([64, 8, 7, 7], F32, tag="ep0")
                nc.vector.tensor_add(t0[:], p0g[:, :, 1:8, 1:8], u0[:])
                t1 = work.tile([64, 8, 7, 7], F32, tag="ep1")
                nc.vector.tensor_add(t1[:], p1g[:, :, 1:8, 1:8], u1[:])
                t2 = work.tile([64, 8, 7, 7], F32, tag="ep2")
                nc.vector.tensor_add(t2[:], t0[:], t1[:])
                t3 = work.tile([64, 8, 7, 7], F32, tag="ep")
                nc.vector.tensor_scalar_max(t3[:], t2[:], 0.0)
                red = work.tile([64, 8], F32, tag="red")
                nc.vector.tensor_reduce(red[:], t3[:],
                                        axis=mybir.AxisListType.XY,
                                        op=ALU.add)
                feats = work.tile([80, 8], MR, tag="feats")
                nc.scalar.activation(feats[0:64, :], red[:], AF.Identity,
                                     bias=svf("pshift"), scale=svf("pscale"))

                # an: relu(an_w [s2; feats] + an_b), one K=80 matmul per half
                gcol = slice(g * FPG, (g + 1) * FPG)
                nc.gpsimd.tensor_copy(feats[64:80, :], s2[:, gcol])
                for mc in range(2):
                    pan = psr.tile([128, FPG], F32, tag="ps")
                    mm(pan[:], svc("anT", mc * 128, 128), feats[:],
                       start=True, stop=True)
                    nc.scalar.activation(s_enc[:, mc, gcol], pan[:], AF.Relu,
                                         bias=svcf("anb", mc, 1))

                # encoder steps that become ready after this group
                eprz, epni = enc_preload(g)
                for ti in range(2):
                    whh_mms(eprz[:, ti, :], epni[:, ti, :], h_cur)
                    h_cur = cell_chain(eprz[:, ti, :], epni[:, ti, :], h_cur)

            # ---- decoder ----
            # Emission order puts everything that depends only on hn(t-1)
            # (whh parts) ahead of the xr(t-1)-dependent wih work, and
            # defers fn(t-1) behind the whh block, so the tensor queue
            # keeps moving during the fi/xr window.
            def emit_fn(x, tt):
                pfn = psr.tile([2, 4], F32, tag="ps")
                for kc in range(2):
                    mm(pfn[:], fnw[:, kc * 2:(kc + 1) * 2],
                       x[:, kc * 4:(kc + 1) * 4],
                       start=(kc == 0), stop=(kc == 1))
                nc.scalar.activation(preds[:, tt * 4:(tt + 1) * 4], pfn[:],
                                     AF.Tanh, bias=svf("fnb"))

            xi, hh = h_cur, h_cur
            for t in range(HOR):
                dp = gps.tile([128, 5, 16], F32, tag="eg")
                prz, pni = dp[:, 0, :], dp[:, 1, :]
                mm(prz, sv("brz4"), sv("sel16"), start=True, stop=False)
                mm(pni, sv("bni4"), sv("selni"), start=False, stop=False)
                pniv = pni.rearrange("p (c k) -> p c k", c=2)
                # whh parts (ready at hh)
                for mc in range(4):
                    reg = prz[:, mc * 4:(mc + 1) * 4]
                    for kc in range(2):
                        mm(reg, whh[:, kc * 768 + mc * 128:
                                    kc * 768 + (mc + 1) * 128],
                           hh[:, kc * 4:(kc + 1) * 4],
                           start=False, stop=False)
                for mc2 in range(2):
                    reg = pniv[:, mc2, 0:8:2]
                    for kc in range(2):
                        mm(reg, whh[:, kc * 768 + (4 + mc2) * 128:
                                    kc * 768 + (5 + mc2) * 128],
                           hh[:, kc * 4:(kc + 1) * 4],
                           start=False, stop=False)
                if t > 0:
                    emit_fn(xi, t - 1)
                # wih parts (ready at xi)
                mm(dp[:, 2, :], sv("brz4"), sv("sel16"),
                   start=False, stop=True)
                for mc in range(4):
                    reg = prz[:, mc * 4:(mc + 1) * 4]
                    for kc in range(2):
                        mm(reg, wih[:, kc * 768 + mc * 128:
                                    kc * 768 + (mc + 1) * 128],
                           xi[:, kc * 4:(kc + 1) * 4],
                           start=False, stop=(mc == 3 and kc == 1))
                for mc2 in range(2):
                    reg = pniv[:, mc2, 1:8:2]
                    for kc in range(2):
                        mm(reg, wih[:, kc * 768 + (4 + mc2) * 128:
                                    kc * 768 + (5 + mc2) * 128],
                           xi[:, kc * 4:(kc + 1) * 4],
                           start=False, stop=(mc2 == 1 and kc == 1))
                # fi bias rides the idle tensor window before hn is ready
                pfi = psr.tile([128, 8], F32, tag="ps")
                mm(pfi[:], sv("fib2"), sv("sel8"), start=True, stop=False)

                hn = cell_chain(prz, pni, hh)

                # final_i residual: xr = hn + relu(fi hn + fi_b)
                for mc2 in range(2):
                    reg = pfi[:, mc2 * 4:(mc2 + 1) * 4]
                    for kc2 in range(2):
                        mm(reg, fiw[:, kc2 * 256 + mc2 * 128:
                                    kc2 * 256 + (mc2 + 1) * 128],
                           hn[:, kc2 * 4:(kc2 + 1) * 4],
                           start=False, stop=(mc2 == 1 and kc2 == 1))
                xr = hpool.tile([128, 8], MR, tag="xr")
                nc.vector.scalar_tensor_tensor(
                    xr[:], pfi[:], 0.0, hn,
                    op0=ALU.max, op1=ALU.add)
                xi, hh = xr[:], hn
            emit_fn(xi, HOR - 1)

            nc.sync.dma_start(h_out[:], preds[:])

    nc.finalize()
    return nc


# ---------------- host-side data prep ----------------

def _prep_frames(frames):
    """frames (32,16,3,112,112) -> per-core [NG, 128, 6*FW] gutter-layout
    patch-T fp8 (8x8 cell grid per frame, row0/col0 + LEAD/TAIL zeros)."""
    out = np.empty((NCORES, NG, 128, 6 * FW), mybir.dt.np(F8))
    fr = np.ascontiguousarray(frames, np.float32)
    for c in range(NCORES):
        fb = fr[c * BPC:(c + 1) * BPC]  # (4, 16, 3, 112, 112)
        a = fb.reshape(BPC, L, 3, 7, 16, 7, 16)
        # -> [l, b, ch, kh, kw, ph, pw]
        a = a.transpose(1, 0, 2, 4, 6, 3, 5)
        a = a.reshape(L, BPC, 768, 49)
        a = a.reshape(NG, 2, BPC, 6, 128, 49)
        # -> [g, k, p, li, b, s]
        a = a.transpose(0, 3, 4, 1, 2, 5)
        a = a.reshape(NG, 6, 128, 8, 7, 7)
        buf = np.zeros((NG, 6, 128, FW), np.float32)
        grid = buf[:, :, :, LEAD:LEAD + 512].reshape(NG, 6, 128, 8, 8, 8)
        grid[:, :, :, :, 1:8, 1:8] = a
        out[c] = buf.transpose(0, 2, 1, 3).reshape(
            NG, 128, 6 * FW).astype(mybir.dt.np(F8))
    return out


def _tap_weights(iv):
    """Composite per-tap weights W_t = cnn1_w[:,:,dh,dw] @ W1 (64, 768),
    keyed by gutter-slot offset (dh-1)*8 + (dw-1), scaled by SC."""
    W1f = iv["cnn_w"].reshape(576, 768).astype(np.float32)
    T = {}
    for dh in range(3):
        for dw in range(3):
            off = (dh - 1) * 8 + (dw - 1)
            T[off] = (iv["cnn1_w"][:, :, dh, dw].astype(np.float32)
                      @ W1f) * SC
    return T


def _prep_weights(iv):
    w = {}
    f8 = mybir.dt.np(F8)
    T = _tap_weights(iv)

    # composite conv lhsT blocks, in matmul emission order:
    # 9x [128, 2, 128] P0 pair blocks, 3x [128, 2, 64] solo-tap blocks,
    # 3x [128, 2, 128] P1 pair blocks.
    wcb = np.zeros((128, 3456), np.float32)
    col = 0
    for pair in P0_PAIRS + [None, P1_PAIR]:
        for q in range(3):
            if pair is None:
                blk = np.zeros((128, 2, 64), np.float32)
                for j in range(2):
                    c = 2 * q + j
                    blk[:, j, :] = T[SOLO_TAP][:, c * 128:(c + 1) * 128].T
                wcb[:, col:col + 128] = blk.reshape(128, 128)
                col += 128
            else:
                lo, up = pair
                blk = np.zeros((128, 2, 128), np.float32)
                for j in range(2):
                    c = 2 * q + j
                    blk[:, j, 0:64] = T[lo][:, c * 128:(c + 1) * 128].T
                    blk[:, j, 64:128] = T[up][:, c * 128:(c + 1) * 128].T
                wcb[:, col:col + 256] = blk.reshape(128, 256)
                col += 256
    assert col == 3456
    w["wc"] = np.ascontiguousarray(wcb).astype(f8).view(np.float32)

    rdt = mybir.dt.np(_dt_of(MM_DT_RNN))
    for name, key in (("wih", "w_ih"), ("whh", "w_hh")):
        T = iv[key].T.astype(np.float32)  # (256, 768)
        w[name] = np.ascontiguousarray(
            T.reshape(2, 128, 768).transpose(1, 0, 2).reshape(
                128, 1536)).astype(rdt)
    T = iv["fi_w"].T.astype(np.float32)  # (256, 256)
    w["fiw"] = np.ascontiguousarray(
        T.reshape(2, 128, 256).transpose(1, 0, 2).reshape(128, 512)).astype(rdt)
    T = iv["fn_w"].T.astype(np.float32)  # (256, 2)
    w["fnw"] = np.ascontiguousarray(
        T.reshape(2, 128, 2).transpose(1, 0, 2).reshape(128, 4)).astype(rdt)
    return w


def _prep_smalls(iv, x, core):
    smm = np.zeros((128, SMM_COLS), mybir.dt.np(_dt_of(MM_DT_RNN)))
    smb = np.zeros((128, SMB_COLS), np.float32)

    def put(name, arr):
        if name in SMM_LAYOUT:
            r0, rows, off, width = SMM_LAYOUT[name]
            dst = smm
        else:
            r0, rows, off, width = SMB_LAYOUT[name]
            dst = smb
        a = np.asarray(arr, np.float32).reshape(rows, width)
        dst[r0:r0 + rows, off:off + width] = a.astype(dst.dtype)

    # composite conv bias = cnn1_b + sum over in-range taps of
    # (W2_t @ cnn_b): rank-10 basis (bcT) x 0/1 validity patterns (bcP),
    # folded into PSUM by one matmul; x SC to match the psum scale
    M = np.einsum("oiab,i->oab", iv["cnn1_w"], iv["cnn_b"]).astype(np.float32)
    bct = np.zeros((10, 64), np.float32)
    bcp = np.zeros((10, 512), np.float32)
    bct[0] = iv["cnn1_b"].astype(np.float32)
    bcp[0] = 1.0
    grid = bcp.reshape(10, 8, 8, 8)
    ti = 1
    for dh in range(3):
        for dw in range(3):
            bct[ti] = M[:, dh, dw]
            for r in range(8):
                for cc in range(8):
                    if 2 <= r + dh <= 8 and 2 <= cc + dw <= 8:
                        grid[ti, :, r, cc] = 1.0
            ti += 1
    put("bcT", bct * SC)
    put("bcP", bcp)

    inv = iv["bn_g"] / np.sqrt(iv["bn_v"] + BN_EPS)
    put("pscale", (inv / 49.0 / SC)[:, None])
    put("pshift", (iv["bn_b"] - iv["bn_m"] * inv)[:, None])

    xb = x[core * BPC:(core + 1) * BPC]  # (4, 16, 12)
    put("xt", xb.transpose(2, 1, 0).reshape(12, 64))

    put("a0t", iv["a0_w"].T)
    put("a0b", iv["a0_b"][:, None])
    put("ait", iv["ai_w"].T)
    put("aib", iv["ai_b"][:, None])
    put("anT", np.concatenate([iv["an_w"][:, 16:80].T,
                               iv["an_w"][:, 0:16].T], axis=0))
    put("anb", iv["an_b"].reshape(2, 128).T)

    put("fnb", iv["fn_b"][:, None])
    bs = (iv["b_ih"] + iv["b_hh"]).astype(np.float32)
    put("brz4", bs[:512].reshape(4, 128))
    # n-gate interleaved bias: rows [bhh c0, bhh c1, bih c0, bih c1];
    # slot j = c*8 + b*2 + parity -> row parity*2 + c
    put("bni4", np.concatenate([iv["b_hh"][512:].reshape(2, 128),
                                iv["b_ih"][512:].reshape(2, 128)]))
    selni = np.zeros((4, 16), np.float32)
    for j in range(16):
        selni[(j % 2) * 2 + j // 8, j] = 1.0
    put("selni", selni)
    put("fib2", iv["fi_b"].reshape(2, 128))
    put("sel16", np.repeat(np.eye(4, dtype=np.float32), 4, axis=1))
    put("sel8", np.repeat(np.eye(2, dtype=np.float32), 4, axis=1))
    return smm, smb


def make_in_maps(inputs):
    iv = {k: np.asarray(v, np.float32) for k, v in inputs.items()}
    frames = iv["frames"]
    x = iv["x"]
    fr_all = _prep_frames(frames)
    w = _prep_weights(iv)
    in_maps = []
    for c in range(NCORES):
        smm, smb = _prep_smalls(iv, x, c)
        m = {"fr": np.ascontiguousarray(fr_all[c]).view(np.float32),
             "smm": smm, "smb": smb}
        m.update(w)
        in_maps.append(m)
    return in_maps


_NC_CACHE = None


def get_nc():
    global _NC_CACHE
    if _NC_CACHE is None:
        _NC_CACHE = build_nc()
    return _NC_CACHE


def _install_ntff_hook():
    """The agent image's antenv lacks axon_hooks; synthesize it so
    run_bass_kernel_spmd(trace=True) can capture NTFF profiles."""
    try:
        from antenv.axon_hooks import get_axon_ntff_profile_hook  # noqa: F401
        return True
    except ImportError:
        pass
    try:
        import types
        import antenv
        if "/root/.axon_site" not in sys.path:
            sys.path.insert(0, "/root/.axon_site")
        from trn_agent_boot.trn_boot import _ntff_profile_via_ctypes
        hook = _ntff_profile_via_ctypes("/opt/axon/libaxon_pjrt.so")
        mod = types.ModuleType("antenv.axon_hooks")
        mod.get_axon_ntff_profile_hook = lambda: hook
        mod.set_axon_ntff_profile_hook = lambda h: None
        sys.modules["antenv.axon_hooks"] = mod
        antenv.axon_hooks = mod
        return hook is not None
    except Exception as e:  # pragma: no cover - profiling is best-effort
        print(f"ntff hook install failed: {e}")
        return False


def kernel(**inputs):
    global LAST_EXEC_NS, LAST_RESULTS
    nc = get_nc()
    in_maps = make_in_maps(inputs)
    trace = bool(int(os.environ.get("KERNEL_TRACE", "0")))
    if trace:
        trace = _install_ntff_hook()
    res = run_bass_kernel_spmd(nc, in_maps, core_ids=list(range(NCORES)),
                               trace=trace)
    LAST_RESULTS = res
    LAST_EXEC_NS = res.exec_time_ns
    outs = []
    for c in range(NCORES):
        o = res.results[c]["out"]  # (2, 40)
        outs.append(o.reshape(2, HOR, BPC).transpose(1, 2, 0)[:, :, None, :])
    return np.concatenate(outs, axis=1).astype(np.float32)


if __name__ == "__main__":
    nc = get_nc()
    print("built ok; instructions:",
          sum(len(bb.instructions) for bb in nc.main_func.blocks))



# revision 48
# speedup vs baseline: 1.0671x; 1.0135x over previous
"""CRNN Trainium2 kernel: patchify-conv -> 3x3 conv -> pool -> GRU encoder ->
autoregressive GRU decoder. Pure data-parallel over batch (32 -> 8 cores x 4).

v3: composite conv. conv1 (patchify) and conv2 (3x3) have no nonlinearity
between them, so they fold into 9 tap weights W_t = W2_t @ W1 applied
directly to the fp8 patch buffer (gutter layout, zeros baked host-side).
Tap pairs are M-packed into the 128 PE output partitions with a constant
slot shift between the two halves (delta -8 rows / -1 col), so the whole
conv stack is 15 DoubleRow matmuls + 1 bias matmul per 8-frame group
(was 38). Epilogue combines the shifted PSUM halves with 3 DVE adds.
"""

import os
import sys

for _p in ("/opt/trn_rl_repo", "/root/.axon_site/_ro/trn_rl_repo"):
    if os.path.isdir(_p) and _p not in sys.path:
        sys.path.insert(0, _p)

import numpy as np

import concourse.bass as bass  # noqa: E402
import concourse.mybir as mybir  # noqa: E402
import concourse.tile as tile  # noqa: E402
from concourse import bacc  # noqa: E402
from concourse.bass_utils import run_bass_kernel_spmd  # noqa: E402

F32 = mybir.dt.float32
F8 = mybir.dt.float8e4
AF = mybir.ActivationFunctionType
ALU = mybir.AluOpType
DR = mybir.MatmulPerfMode.DoubleRow

# Model dims (hardcoded from the problem spec)
B, L, DS, DA, DC, DRN, DO, HOR = 32, 16, 12, 16, 64, 256, 2, 10
NCORES, BPC = 8, 4          # batch per core
NG, FPG = 8, 8              # 8 groups of 8 frames per core (frame idx = l*4+b)
BN_EPS = 1e-5
SC = 32.0                   # fp8 composite conv weight scale
LEAD, FW = 16, 16 + 8 * 64 + 16   # 544-col gutter row per patch K-chunk
# composite conv M-pack: P0 pairs (lower off, upper off) share delta -8;
# tap +9 runs solo into P0's lower half; P1 holds the delta -1 pair.
P0_PAIRS = [(-1, -9), (0, -8), (1, -7)]
P1_PAIR = (8, 7)
SOLO_TAP = 9

MM_DT_RNN = os.environ.get("BASS_MM_DT_RNN", "f16")


def _dt_of(tag):
    return {"f32": mybir.dt.float32, "f32r": mybir.dt.float32r,
            "bf16": mybir.dt.bfloat16, "f16": mybir.dt.float16}[tag]

LAST_EXEC_NS = None
LAST_RESULTS = None


def _layout(entries):
    """entries: (name, rows, width[, row0]) -> dict + total cols."""
    out = {}
    cols = 0
    for e in entries:
        name, rows, width = e[0], e[1], e[2]
        row0 = e[3] if len(e) > 3 else 0
        out[name] = (row0, rows, cols, width)
        cols += width
    return out, cols


# matmul operands (RNN matmul dtype)
SMM_LAYOUT, SMM_COLS = _layout([
    ("xt", 12, 64),          # per-core x transposed, col = l*4+b
    ("a0t", 12, 16),
    ("ait", 16, 16),
    ("anT", 80, 256),        # [an_w[:,16:80].T ; an_w[:,0:16].T] rows
    # composite conv bias: rank-10 basis x 0/1 tap-validity patterns
    ("bcT", 10, 64),         # [cnn1_b ; W2_t @ cnn_b per tap] rows
    ("bcP", 10, 512),        # per-slot validity patterns (tiled x8 frames)
    # gate bias-into-psum operands: one matmul per psum tile
    # (lhsT = bias chunks as rows, rhs = chunk->column selector)
    ("brz4", 4, 128),        # (b_ih+b_hh) rz chunks
    ("bni4", 4, 128),        # [b_hh n chunks ; b_ih n chunks] rows
    ("fib2", 2, 128),        # fi_b chunks
    ("sel16", 4, 16),        # selector: col n lights chunk n//4
    ("selni", 4, 16),        # interleaved: even col j -> bhh j//8,
                             # odd col j -> bih j//8
    ("sel8", 2, 8),
])
# bias/affine tables (always fp32)
SMB_LAYOUT, SMB_COLS = _layout([
    ("pscale", 64, 1),       # inv/49/SC
    ("pshift", 64, 1),
    ("a0b", 16, 1),
    ("aib", 16, 1),
    ("anb", 128, 2),         # an_b chunks as cols
    ("fnb", 2, 1),
])

def build_nc():
    nc = bacc.Bacc("TRN2", target_bir_lowering=False, debug=False,
                   num_devices=NCORES)
    mm_rnn = _dt_of(MM_DT_RNN)
    MR = mm_rnn

    # big tensors are DMA'd bitcast to f32: the DMA engines are
    # element-rate limited, so 4x fewer elements = ~4x the bandwidth
    h_fr = nc.dram_tensor("fr", [NG, 128, 6 * FW // 4], F32,
                          kind="ExternalInput")
    h_smm = nc.dram_tensor("smm", [128, SMM_COLS], MR, kind="ExternalInput")
    h_smb = nc.dram_tensor("smb", [128, SMB_COLS], F32, kind="ExternalInput")
    h_wc = nc.dram_tensor("wc", [128, 864], F32, kind="ExternalInput")
    h_wih = nc.dram_tensor("wih", [128, 2 * 768], MR, kind="ExternalInput")
    h_whh = nc.dram_tensor("whh", [128, 2 * 768], MR, kind="ExternalInput")
    h_fi = nc.dram_tensor("fiw", [128, 2 * 256], MR, kind="ExternalInput")
    h_fn = nc.dram_tensor("fnw", [128, 4], MR, kind="ExternalInput")
    h_out = nc.dram_tensor("out", [2, 4 * HOR], F32, kind="ExternalOutput")

    def mm(out, lhsT, rhs, **kw):
        nc.tensor.matmul(out, lhsT, rhs, skip_group_check=True, **kw)

    with tile.TileContext(nc) as tc:
        from contextlib import ExitStack
        with ExitStack() as ctx:
            cpool = ctx.enter_context(tc.tile_pool(name="const", bufs=1))
            work = ctx.enter_context(tc.tile_pool(name="work", bufs=4))
            state = ctx.enter_context(tc.tile_pool(name="state", bufs=1))
            hpool = ctx.enter_context(tc.tile_pool(name="h", bufs=3))
            cps = ctx.enter_context(
                tc.tile_pool(name="cps", bufs=2, space="PSUM"))
            gps = ctx.enter_context(
                tc.tile_pool(name="gps", bufs=2, space="PSUM"))
            psr = ctx.enter_context(
                tc.tile_pool(name="psr", bufs=2, space="PSUM"))

            # ---- constants + frames to SBUF ----
            # Sync queue: g0 frames, conv weights, then the remaining
            # frame groups back-to-back (descriptor issue is ~0.6us each,
            # so one start per group, all resident). Scalar queue: the
            # small RNN weights, done well before first use.
            xins = []
            for g in range(NG):
                xin_g = cpool.tile([128, 6, FW], F8, tag=f"xin{g}")
                xins.append(xin_g)

            def fr_dma(g):
                nc.sync.dma_start(
                    xins[g][:].rearrange("p a b -> p (a b)").bitcast(F32),
                    h_fr[g])

            fr_dma(0)
            wc = cpool.tile([128, 3456], F8, tag="wc")
            nc.sync.dma_start(wc[:].bitcast(F32), h_wc[:])
            for g in range(1, NG):
                fr_dma(g)
            smm = cpool.tile([128, SMM_COLS], MR, tag="smm")
            nc.scalar.dma_start(smm[:], h_smm[:])
            smb = cpool.tile([128, SMB_COLS], F32, tag="smb")
            nc.scalar.dma_start(smb[:], h_smb[:])
            wih = cpool.tile([128, 2 * 768], MR, tag="wih")
            nc.scalar.dma_start(wih[:].bitcast(F32), h_wih[:].bitcast(F32))
            whh = cpool.tile([128, 2 * 768], MR, tag="whh")
            nc.scalar.dma_start(whh[:].bitcast(F32), h_whh[:].bitcast(F32))
            fiw = cpool.tile([128, 2 * 256], MR, tag="fiw")
            nc.scalar.dma_start(fiw[:].bitcast(F32), h_fi[:].bitcast(F32))
            fnw = cpool.tile([128, 4], MR, tag="fnw")
            nc.scalar.dma_start(fnw[:], h_fn[:])

            def sv(name):  # matmul-operand view (RNN dtype)
                r0, rows, off, width = SMM_LAYOUT[name]
                return smm[r0:r0 + rows, off:off + width]

            def svc(name, c0, w):
                r0, rows, off, width = SMM_LAYOUT[name]
                assert c0 + w <= width
                return smm[r0:r0 + rows, off + c0:off + c0 + w]

            def svf(name):  # fp32 bias/affine view
                r0, rows, off, width = SMB_LAYOUT[name]
                return smb[r0:r0 + rows, off:off + width]

            def svcf(name, c0, w):
                r0, rows, off, width = SMB_LAYOUT[name]
                assert c0 + w <= width
                return smb[r0:r0 + rows, off + c0:off + c0 + w]

            # PE warm-up: the tensor engine p-state ramps only under load,
            # and the first real matmuls otherwise run ~2.5x slow. Burn the
            # DMA-wait window (~2.5-10us) with throwaway matmuls on a
            # zeroed tile.
            wtile = work.tile([128, 512], F8, tag="warm")
            nc.vector.memset(wtile[:], 0.0)
            pw = psr.tile([64, 512], F32, tag="ps")
            for wi in range(16):
                mm(pw[:], wtile[:, 0:64], wtile[:, 0:512],
                   start=True, stop=True)

            # ---- persistent state tiles ----
            s2 = state.tile([16, 64], MR, tag="s2")
            s_enc = state.tile([128, 2, 64], MR, tag="senc")
            preds = state.tile([2, 4 * HOR], F32, tag="preds")
            # GRU scan operand tiles (even slots stay zero forever; odd
            # slots rewritten each step). Layout: slot 2i(+1) with pair
            # i = (chunk i//4, batch i%4) matching gate psum column order.
            rzscan = state.tile([128, 32], F32, tag="rzscan")
            nc.vector.memset(rzscan[:], 0.0)
            ozscan = state.tile([128, 16], F32, tag="ozscan")
            nc.gpsimd.memset(ozscan[:], 0.0)
            tzscan = state.tile([128, 16], F32, tag="tzscan")
            nc.gpsimd.memset(tzscan[:], 0.0)
            rz2 = rzscan[:].rearrange("p (a b) -> p a b", b=2)
            oz2 = ozscan[:].rearrange("p (a b) -> p a b", b=2)
            tz2 = tzscan[:].rearrange("p (a b) -> p a b", b=2)

            # ---- state adapters: s1 = relu(a0 x); s2 = s1 + relu(ai s1) ----
            # Emitted mid-group-0 so the conv1 matmuls (which only need
            # w1+xin0) lead the tensor queue instead of stalling on smm.
            def emit_adapters():
                pa = psr.tile([16, 64], F32, tag="ps")
                mm(pa[:], sv("a0t"), sv("xt"), start=True, stop=True)
                s1 = work.tile([16, 64], MR, tag="s1")
                nc.scalar.activation(s1[:], pa[:], AF.Relu, bias=svf("a0b"))
                pb = psr.tile([16, 64], F32, tag="ps")
                mm(pb[:], sv("ait"), s1[:], start=True, stop=True)
                s1b = work.tile([16, 64], MR, tag="s1")
                nc.scalar.activation(s1b[:], pb[:], AF.Relu, bias=svf("aib"))
                nc.vector.tensor_add(s2[:], s1[:], s1b[:])

            # force the combined activation table load at startup (hidden
            # in the DMA wait) instead of a 1.3us stall at the first
            # sigmoid: touch every function used, rarest first
            dum = work.tile([1, 4], F32, tag="dum")
            for fn in (AF.Sigmoid, AF.Tanh, AF.Relu, AF.Identity, AF.Copy):
                nc.scalar.activation(dum[:], rzscan[0:1, 0:4], fn)

            # encoder hidden state: odd slots of a scan-layout tile
            h0 = hpool.tile([128, 16], MR, tag="h")
            nc.gpsimd.memset(h0[:], 0.0)
            h_cur = h0[:].rearrange("p (a b) -> p a b", b=2)[:, :, 1]

            def whh_mms(prz16, pni16, hv):
                """whh gate matmuls for one step (the only mms after h)."""
                for mc in range(4):
                    reg = prz16[:, mc * 4:(mc + 1) * 4]
                    for kc in range(2):
                        mm(reg, whh[:, kc * 768 + mc * 128:
                                    kc * 768 + (mc + 1) * 128],
                           hv[:, kc * 4:(kc + 1) * 4],
                           start=False, stop=(mc == 3 and kc == 1))
                pniv = pni16.rearrange("p (c k) -> p c k", c=2)
                for mc2 in range(2):
                    reg = pniv[:, mc2, 0:8:2]
                    for kc in range(2):
                        mm(reg, whh[:, kc * 768 + (4 + mc2) * 128:
                                    kc * 768 + (5 + mc2) * 128],
                           hv[:, kc * 4:(kc + 1) * 4],
                           start=False, stop=(mc2 == 1 and kc == 1))

            def cell_chain(prz16, pni16, hv, after_sigma=None):
                """sigma -> scan(r*ghn+gin) -> tanh -> scan((1-z)n+zh).
                Returns the new hidden as an odd-slot view."""
                nc.scalar.activation(rz2[:, :, 1], prz16, AF.Sigmoid)
                if after_sigma is not None:
                    after_sigma()
                zv = rz2[:, 8:16, 1]
                nc.gpsimd.tensor_scalar(oz2[:, :, 1], zv, -1.0, 1.0,
                                        op0=ALU.mult, op1=ALU.add)
                nc.gpsimd.tensor_mul(tz2[:, :, 1], zv, hv)
                ns = work.tile([128, 16], F32, tag="nscan")
                nc.vector.tensor_tensor_scan(ns[:], rzscan[:, 0:16], pni16,
                                             0.0, op0=ALU.mult, op1=ALU.add)
                nc.scalar.activation(
                    tz2[:, :, 0],
                    ns[:].rearrange("p (a b) -> p a b", b=2)[:, :, 1],
                    AF.Tanh)
                hs = hpool.tile([128, 16], MR, tag="h")
                nc.vector.tensor_tensor_scan(hs[:], ozscan[:], tzscan[:],
                                             0.0, op0=ALU.mult, op1=ALU.add)
                return hs[:].rearrange("p (a b) -> p a b", b=2)[:, :, 1]

            def enc_preload(g):
                """Per-group gate psum tiles for steps 2g, 2g+1: biases +
                wih @ s_enc land before h is even known."""
                # start=True only on the bank's first mm: a start marks the
                # WHOLE psum bank pending-zero, so later first-writes of
                # other regions zero-fill implicitly (start=False).
                ep = gps.tile([128, 5, 16], F32, tag="eg")
                prz, pni = ep[:, 0:2, :], ep[:, 2:4, :]
                for ti in range(2):
                    mm(prz[:, ti, :], sv("brz4"), sv("sel16"),
                       start=(ti == 0), stop=False)
                    mm(pni[:, ti, :], sv("bni4"), sv("selni"),
                       start=False, stop=False)
                # pad write: clears the 16 elements past pni so CoreSim's
                # strided zero-region window never sees mixed state
                mm(ep[:, 4, :], sv("brz4"), sv("sel16"),
                   start=False, stop=True)
                pniv = pni.rearrange("p s (c k) -> p s c k", c=2)
                for ti in range(2):
                    xs = [s_enc[:, kc, g * FPG + ti * 4:g * FPG + ti * 4 + 4]
                          for kc in range(2)]
                    for mc in range(4):
                        reg = prz[:, ti, mc * 4:(mc + 1) * 4]
                        for kc in range(2):
                            mm(reg, wih[:, kc * 768 + mc * 128:
                                        kc * 768 + (mc + 1) * 128],
                               xs[kc], start=False, stop=False)
                    for mc2 in range(2):
                        reg = pniv[:, ti, mc2, 1:8:2]
                        for kc in range(2):
                            mm(reg, wih[:, kc * 768 + (4 + mc2) * 128:
                                        kc * 768 + (5 + mc2) * 128],
                               xs[kc], start=False,
                               stop=(mc2 == 1 and kc == 1))
                return prz, pni

            # adapters run in the conv-g0 DMA-wait window: smm lands
            # ~10us, well before the first conv matmul needs the PE
            emit_adapters()

            # ---- composite conv + features, per group of 8 frames ----
            for g in range(NG):
                xin = xins[g]
                pc = cps.tile([128, 2, 512], F32, tag="cps")
                P0, P1 = pc[:, 0, :], pc[:, 1, :]

                def rhs(off, q):
                    a = LEAD + off
                    return xin[:, 2 * q:2 * q + 2, a:a + 512]

                def wcv(blk, m):  # block at col 256*?: [128, 2, m]
                    return wc[:, blk:blk + 2 * m].rearrange(
                        "p (a m) -> p a m", a=2)

                # P0: first pair mm starts the accumulation over all 128
                # rows; bias + solo-tap (M=64) mms ride in the middle; the
                # last pair mm closes the group.
                for p, (lo, up) in enumerate(P0_PAIRS):
                    for q in range(3):
                        first = (p == 0 and q == 0)
                        last = (p == 2 and q == 2)
                        mm(P0, wcv((p * 3 + q) * 256, 128), rhs(lo, q),
                           start=first, stop=last, perf_mode=DR)
                        if first:
                            # conv bias (rank-10 basis x validity patterns)
                            mm(P0[0:64], sv("bcT"), sv("bcP"),
                               start=False, stop=False)
                            for q2 in range(3):
                                mm(P0[0:64], wcv(2304 + q2 * 128, 64),
                                   rhs(SOLO_TAP, q2),
                                   start=False, stop=False, perf_mode=DR)
                for q in range(3):
                    mm(P1, wcv(2688 + q * 256, 128), rhs(P1_PAIR[0], q),
                       start=(q == 0), stop=(q == 2), perf_mode=DR)

                # epilogue: combine shifted psum halves -> relu -> sum ->
                # affine. upper halves hold the paired tap accumulated at
                # slot+delta (P0 delta -8 = one grid row, P1 delta -1).
                p0g = pc[0:64, 0, :].rearrange("p (f a b) -> p f a b",
                                               a=8, b=8)
                p0u = pc[64:128, 0, :].rearrange("p (f a b) -> p f a b",
                                                 a=8, b=8)
                p1g = pc[0:64, 1, :].rearrange("p (f a b) -> p f a b",
                                               a=8, b=8)
                p1u = pc[64:128, 1, :].rearrange("p (f a b) -> p f a b",
                                                 a=8, b=8)
                # DVE/ACT ops cannot read two PSUM operands in one
                # instruction: stage the upper halves through SBUF.
                u0 = work.tile([64, 8, 7, 7], F32, tag="epu0")
                nc.scalar.activation(u0[:], p0u[:, :, 0:7, 1:8], AF.Copy)
                u1 = work.tile([64, 8, 7, 7], F32, tag="epu1")
                nc.scalar.activation(u1[:], p1u[:, :, 1:8, 0:7], AF.Copy)
                t0 = work.tile([64, 8, 7, 7], F32, tag="ep0")
                nc.vector.tensor_add(t0[:], p0g[:, :, 1:8, 1:8], u0[:])
                t1 = work.tile([64, 8, 7, 7], F32, tag="ep1")
                nc.vector.tensor_add(t1[:], p1g[:, :, 1:8, 1:8], u1[:])
                t2 = work.tile([64, 8, 7, 7], F32, tag="ep2")
                nc.vector.tensor_add(t2[:], t0[:], t1[:])
                t3 = work.tile([64, 8, 7, 7], F32, tag="ep")
                nc.vector.tensor_scalar_max(t3[:], t2[:], 0.0)
                red = work.tile([64, 8], F32, tag="red")
                nc.vector.tensor_reduce(red[:], t3[:],
                                        axis=mybir.AxisListType.XY,
                                        op=ALU.add)
                feats = work.tile([80, 8], MR, tag="feats")
                nc.scalar.activation(feats[0:64, :], red[:], AF.Identity,
                                     bias=svf("pshift"), scale=svf("pscale"))

                # an: relu(an_w [s2; feats] + an_b), one K=80 matmul per half
                gcol = slice(g * FPG, (g + 1) * FPG)
                nc.gpsimd.tensor_copy(feats[64:80, :], s2[:, gcol])
                for mc in range(2):
                    pan = psr.tile([128, FPG], F32, tag="ps")
                    mm(pan[:], svc("anT", mc * 128, 128), feats[:],
                       start=True, stop=True)
                    nc.scalar.activation(s_enc[:, mc, gcol], pan[:], AF.Relu,
                                         bias=svcf("anb", mc, 1))

                # encoder steps that become ready after this group
                eprz, epni = enc_preload(g)
                for ti in range(2):
                    whh_mms(eprz[:, ti, :], epni[:, ti, :], h_cur)
                    h_cur = cell_chain(eprz[:, ti, :], epni[:, ti, :], h_cur)

            # ---- decoder ----
            # Emission order puts everything that depends only on hn(t-1)
            # (whh parts) ahead of the xr(t-1)-dependent wih work, and
            # defers fn(t-1) behind the whh block, so the tensor queue
            # keeps moving during the fi/xr window.
            def emit_fn(x, tt):
                pfn = psr.tile([2, 4], F32, tag="ps")
                for kc in range(2):
                    mm(pfn[:], fnw[:, kc * 2:(kc + 1) * 2],
                       x[:, kc * 4:(kc + 1) * 4],
                       start=(kc == 0), stop=(kc == 1))
                nc.scalar.activation(preds[:, tt * 4:(tt + 1) * 4], pfn[:],
                                     AF.Tanh, bias=svf("fnb"))

            xi, hh = h_cur, h_cur
            for t in range(HOR):
                dp = gps.tile([128, 5, 16], F32, tag="eg")
                prz, pni = dp[:, 0, :], dp[:, 1, :]
                mm(prz, sv("brz4"), sv("sel16"), start=True, stop=False)
                mm(pni, sv("bni4"), sv("selni"), start=False, stop=False)
                pniv = pni.rearrange("p (c k) -> p c k", c=2)
                # whh parts (ready at hh)
                for mc in range(4):
                    reg = prz[:, mc * 4:(mc + 1) * 4]
                    for kc in range(2):
                        mm(reg, whh[:, kc * 768 + mc * 128:
                                    kc * 768 + (mc + 1) * 128],
                           hh[:, kc * 4:(kc + 1) * 4],
                           start=False, stop=False)
                for mc2 in range(2):
                    reg = pniv[:, mc2, 0:8:2]
                    for kc in range(2):
                        mm(reg, whh[:, kc * 768 + (4 + mc2) * 128:
                                    kc * 768 + (5 + mc2) * 128],
                           hh[:, kc * 4:(kc + 1) * 4],
                           start=False, stop=False)
                if t > 0:
                    emit_fn(xi, t - 1)
                # wih parts (ready at xi)
                mm(dp[:, 2, :], sv("brz4"), sv("sel16"),
                   start=False, stop=True)
                for mc in range(4):
                    reg = prz[:, mc * 4:(mc + 1) * 4]
                    for kc in range(2):
                        mm(reg, wih[:, kc * 768 + mc * 128:
                                    kc * 768 + (mc + 1) * 128],
                           xi[:, kc * 4:(kc + 1) * 4],
                           start=False, stop=(mc == 3 and kc == 1))
                for mc2 in range(2):
                    reg = pniv[:, mc2, 1:8:2]
                    for kc in range(2):
                        mm(reg, wih[:, kc * 768 + (4 + mc2) * 128:
                                    kc * 768 + (5 + mc2) * 128],
                           xi[:, kc * 4:(kc + 1) * 4],
                           start=False, stop=(mc2 == 1 and kc == 1))
                # fi bias rides the idle tensor window before hn is ready
                pfi = psr.tile([128, 8], F32, tag="ps")
                mm(pfi[:], sv("fib2"), sv("sel8"), start=True, stop=False)

                hn = cell_chain(prz, pni, hh)

                # final_i residual: xr = hn + relu(fi hn + fi_b)
                for mc2 in range(2):
                    reg = pfi[:, mc2 * 4:(mc2 + 1) * 4]
                    for kc2 in range(2):
                        mm(reg, fiw[:, kc2 * 256 + mc2 * 128:
                                    kc2 * 256 + (mc2 + 1) * 128],
                           hn[:, kc2 * 4:(kc2 + 1) * 4],
                           start=False, stop=(mc2 == 1 and kc2 == 1))
                xr = hpool.tile([128, 8], MR, tag="xr")
                nc.vector.scalar_tensor_tensor(
                    xr[:], pfi[:], 0.0, hn,
                    op0=ALU.max, op1=ALU.add)
                xi, hh = xr[:], hn
            emit_fn(xi, HOR - 1)

            nc.sync.dma_start(h_out[:], preds[:])

    nc.finalize()
    return nc


# ---------------- host-side data prep ----------------

def _prep_frames(frames):
    """frames (32,16,3,112,112) -> per-core [NG, 128, 6*FW] gutter-layout
    patch-T fp8 (8x8 cell grid per frame, row0/col0 + LEAD/TAIL zeros)."""
    out = np.empty((NCORES, NG, 128, 6 * FW), mybir.dt.np(F8))
    fr = np.ascontiguousarray(frames, np.float32)
    for c in range(NCORES):
        fb = fr[c * BPC:(c + 1) * BPC]  # (4, 16, 3, 112, 112)
        a = fb.reshape(BPC, L, 3, 7, 16, 7, 16)
        # -> [l, b, ch, kh, kw, ph, pw]
        a = a.transpose(1, 0, 2, 4, 6, 3, 5)
        a = a.reshape(L, BPC, 768, 49)
        a = a.reshape(NG, 2, BPC, 6, 128, 49)
        # -> [g, k, p, li, b, s]
        a = a.transpose(0, 3, 4, 1, 2, 5)
        a = a.reshape(NG, 6, 128, 8, 7, 7)
        buf = np.zeros((NG, 6, 128, FW), np.float32)
        grid = buf[:, :, :, LEAD:LEAD + 512].reshape(NG, 6, 128, 8, 8, 8)
        grid[:, :, :, :, 1:8, 1:8] = a
        out[c] = buf.transpose(0, 2, 1, 3).reshape(
            NG, 128, 6 * FW).astype(mybir.dt.np(F8))
    return out


def _tap_weights(iv):
    """Composite per-tap weights W_t = cnn1_w[:,:,dh,dw] @ W1 (64, 768),
    keyed by gutter-slot offset (dh-1)*8 + (dw-1), scaled by SC."""
    W1f = iv["cnn_w"].reshape(576, 768).astype(np.float32)
    T = {}
    for dh in range(3):
        for dw in range(3):
            off = (dh - 1) * 8 + (dw - 1)
            T[off] = (iv["cnn1_w"][:, :, dh, dw].astype(np.float32)
                      @ W1f) * SC
    return T


def _prep_weights(iv):
    w = {}
    f8 = mybir.dt.np(F8)
    T = _tap_weights(iv)

    # composite conv lhsT blocks, in matmul emission order:
    # 9x [128, 2, 128] P0 pair blocks, 3x [128, 2, 64] solo-tap blocks,
    # 3x [128, 2, 128] P1 pair blocks.
    wcb = np.zeros((128, 3456), np.float32)
    col = 0
    for pair in P0_PAIRS + [None, P1_PAIR]:
        for q in range(3):
            if pair is None:
                blk = np.zeros((128, 2, 64), np.float32)
                for j in range(2):
                    c = 2 * q + j
                    blk[:, j, :] = T[SOLO_TAP][:, c * 128:(c + 1) * 128].T
                wcb[:, col:col + 128] = blk.reshape(128, 128)
                col += 128
            else:
                lo, up = pair
                blk = np.zeros((128, 2, 128), np.float32)
                for j in range(2):
                    c = 2 * q + j
                    blk[:, j, 0:64] = T[lo][:, c * 128:(c + 1) * 128].T
                    blk[:, j, 64:128] = T[up][:, c * 128:(c + 1) * 128].T
                wcb[:, col:col + 256] = blk.reshape(128, 256)
                col += 256
    assert col == 3456
    w["wc"] = np.ascontiguousarray(wcb).astype(f8).view(np.float32)

    rdt = mybir.dt.np(_dt_of(MM_DT_RNN))
    for name, key in (("wih", "w_ih"), ("whh", "w_hh")):
        T = iv[key].T.astype(np.float32)  # (256, 768)
        w[name] = np.ascontiguousarray(
            T.reshape(2, 128, 768).transpose(1, 0, 2).reshape(
                128, 1536)).astype(rdt)
    T = iv["fi_w"].T.astype(np.float32)  # (256, 256)
    w["fiw"] = np.ascontiguousarray(
        T.reshape(2, 128, 256).transpose(1, 0, 2).reshape(128, 512)).astype(rdt)
    T = iv["fn_w"].T.astype(np.float32)  # (256, 2)
    w["fnw"] = np.ascontiguousarray(
        T.reshape(2, 128, 2).transpose(1, 0, 2).reshape(128, 4)).astype(rdt)
    return w


def _prep_smalls(iv, x, core):
    smm = np.zeros((128, SMM_COLS), mybir.dt.np(_dt_of(MM_DT_RNN)))
    smb = np.zeros((128, SMB_COLS), np.float32)

    def put(name, arr):
        if name in SMM_LAYOUT:
            r0, rows, off, width = SMM_LAYOUT[name]
            dst = smm
        else:
            r0, rows, off, width = SMB_LAYOUT[name]
            dst = smb
        a = np.asarray(arr, np.float32).reshape(rows, width)
        dst[r0:r0 + rows, off:off + width] = a.astype(dst.dtype)

    # composite conv bias = cnn1_b + sum over in-range taps of
    # (W2_t @ cnn_b): rank-10 basis (bcT) x 0/1 validity patterns (bcP),
    # folded into PSUM by one matmul; x SC to match the psum scale
    M = np.einsum("oiab,i->oab", iv["cnn1_w"], iv["cnn_b"]).astype(np.float32)
    bct = np.zeros((10, 64), np.float32)
    bcp = np.zeros((10, 512), np.float32)
    bct[0] = iv["cnn1_b"].astype(np.float32)
    bcp[0] = 1.0
    grid = bcp.reshape(10, 8, 8, 8)
    ti = 1
    for dh in range(3):
        for dw in range(3):
            bct[ti] = M[:, dh, dw]
            for r in range(8):
                for cc in range(8):
                    if 2 <= r + dh <= 8 and 2 <= cc + dw <= 8:
                        grid[ti, :, r, cc] = 1.0
            ti += 1
    put("bcT", bct * SC)
    put("bcP", bcp)

    inv = iv["bn_g"] / np.sqrt(iv["bn_v"] + BN_EPS)
    put("pscale", (inv / 49.0 / SC)[:, None])
    put("pshift", (iv["bn_b"] - iv["bn_m"] * inv)[:, None])

    xb = x[core * BPC:(core + 1) * BPC]  # (4, 16, 12)
    put("xt", xb.transpose(2, 1, 0).reshape(12, 64))

    put("a0t", iv["a0_w"].T)
    put("a0b", iv["a0_b"][:, None])
    put("ait", iv["ai_w"].T)
    put("aib", iv["ai_b"][:, None])
    put("anT", np.concatenate([iv["an_w"][:, 16:80].T,
                               iv["an_w"][:, 0:16].T], axis=0))
    put("anb", iv["an_b"].reshape(2, 128).T)

    put("fnb", iv["fn_b"][:, None])
    bs = (iv["b_ih"] + iv["b_hh"]).astype(np.float32)
    put("brz4", bs[:512].reshape(4, 128))
    # n-gate interleaved bias: rows [bhh c0, bhh c1, bih c0, bih c1];
    # slot j = c*8 + b*2 + parity -> row parity*2 + c
    put("bni4", np.concatenate([iv["b_hh"][512:].reshape(2, 128),
                                iv["b_ih"][512:].reshape(2, 128)]))
    selni = np.zeros((4, 16), np.float32)
    for j in range(16):
        selni[(j % 2) * 2 + j // 8, j] = 1.0
    put("selni", selni)
    put("fib2", iv["fi_b"].reshape(2, 128))
    put("sel16", np.repeat(np.eye(4, dtype=np.float32), 4, axis=1))
    put("sel8", np.repeat(np.eye(2, dtype=np.float32), 4, axis=1))
    return smm, smb


def make_in_maps(inputs):
    iv = {k: np.asarray(v, np.float32) for k, v in inputs.items()}
    frames = iv["frames"]
    x = iv["x"]
    fr_all = _prep_frames(frames)
    w = _prep_weights(iv)
    in_maps = []
    for c in range(NCORES):
        smm, smb = _prep_smalls(iv, x, c)
        m = {"fr": np.ascontiguousarray(fr_all[c]).view(np.float32),
             "smm": smm, "smb": smb}
        m.update(w)
        in_maps.append(m)
    return in_maps


_NC_CACHE = None


def get_nc():
    global _NC_CACHE
    if _NC_CACHE is None:
        _NC_CACHE = build_nc()
    return _NC_CACHE


def _install_ntff_hook():
    """The agent image's antenv lacks axon_hooks; synthesize it so
    run_bass_kernel_spmd(trace=True) can capture NTFF profiles."""
    try:
        from antenv.axon_hooks import get_axon_ntff_profile_hook  # noqa: F401
        return True
    except ImportError:
        pass
    try:
        import types
        import antenv
        if "/root/.axon_site" not in sys.path:
            sys.path.insert(0, "/root/.axon_site")
        from trn_agent_boot.trn_boot import _ntff_profile_via_ctypes
        hook = _ntff_profile_via_ctypes("/opt/axon/libaxon_pjrt.so")
        mod = types.ModuleType("antenv.axon_hooks")
        mod.get_axon_ntff_profile_hook = lambda: hook
        mod.set_axon_ntff_profile_hook = lambda h: None
        sys.modules["antenv.axon_hooks"] = mod
        antenv.axon_hooks = mod
        return hook is not None
    except Exception as e:  # pragma: no cover - profiling is best-effort
        print(f"ntff hook install failed: {e}")
        return False


def kernel(**inputs):
    global LAST_EXEC_NS, LAST_RESULTS
    nc = get_nc()
    in_maps = make_in_maps(inputs)
    trace = bool(int(os.environ.get("KERNEL_TRACE", "0")))
    if trace:
        trace = _install_ntff_hook()
    res = run_bass_kernel_spmd(nc, in_maps, core_ids=list(range(NCORES)),
                               trace=trace)
    LAST_RESULTS = res
    LAST_EXEC_NS = res.exec_time_ns
    outs = []
    for c in range(NCORES):
        o = res.results[c]["out"]  # (2, 40)
        outs.append(o.reshape(2, HOR, BPC).transpose(1, 2, 0)[:, :, None, :])
    return np.concatenate(outs, axis=1).astype(np.float32)


if __name__ == "__main__":
    nc = get_nc()
    print("built ok; instructions:",
          sum(len(bb.instructions) for bb in nc.main_func.blocks))



# revision 49
# speedup vs baseline: 1.0681x; 1.0009x over previous
"""CRNN Trainium2 kernel: patchify-conv -> 3x3 conv -> pool -> GRU encoder ->
autoregressive GRU decoder. Pure data-parallel over batch (32 -> 8 cores x 4).

v3: composite conv. conv1 (patchify) and conv2 (3x3) have no nonlinearity
between them, so they fold into 9 tap weights W_t = W2_t @ W1 applied
directly to the fp8 patch buffer (gutter layout, zeros baked host-side).
Tap pairs are M-packed into the 128 PE output partitions with a constant
slot shift between the two halves (delta -8 rows / -1 col), so the whole
conv stack is 15 DoubleRow matmuls + 1 bias matmul per 8-frame group
(was 38). Epilogue combines the shifted PSUM halves with 3 DVE adds.
"""

import os
import sys

for _p in ("/opt/trn_rl_repo", "/root/.axon_site/_ro/trn_rl_repo"):
    if os.path.isdir(_p) and _p not in sys.path:
        sys.path.insert(0, _p)

import numpy as np

import concourse.bass as bass  # noqa: E402
import concourse.mybir as mybir  # noqa: E402
import concourse.tile as tile  # noqa: E402
from concourse import bacc  # noqa: E402
from concourse.bass_utils import run_bass_kernel_spmd  # noqa: E402

F32 = mybir.dt.float32
F8 = mybir.dt.float8e4
AF = mybir.ActivationFunctionType
ALU = mybir.AluOpType
DR = mybir.MatmulPerfMode.DoubleRow

# Model dims (hardcoded from the problem spec)
B, L, DS, DA, DC, DRN, DO, HOR = 32, 16, 12, 16, 64, 256, 2, 10
NCORES, BPC = 8, 4          # batch per core
NG, FPG = 8, 8              # 8 groups of 8 frames per core (frame idx = l*4+b)
BN_EPS = 1e-5
SC = 32.0                   # fp8 composite conv weight scale
LEAD, FW = 16, 16 + 8 * 64 + 16   # 544-col gutter row per patch K-chunk
# composite conv M-pack: P0 pairs (lower off, upper off) share delta -8;
# tap +9 runs solo into P0's lower half; P1 holds the delta -1 pair.
P0_PAIRS = [(-1, -9), (0, -8), (1, -7)]
P1_PAIR = (8, 7)
SOLO_TAP = 9

MM_DT_RNN = os.environ.get("BASS_MM_DT_RNN", "f16")


def _dt_of(tag):
    return {"f32": mybir.dt.float32, "f32r": mybir.dt.float32r,
            "bf16": mybir.dt.bfloat16, "f16": mybir.dt.float16}[tag]

LAST_EXEC_NS = None
LAST_RESULTS = None


def _layout(entries):
    """entries: (name, rows, width[, row0]) -> dict + total cols."""
    out = {}
    cols = 0
    for e in entries:
        name, rows, width = e[0], e[1], e[2]
        row0 = e[3] if len(e) > 3 else 0
        out[name] = (row0, rows, cols, width)
        cols += width
    return out, cols


# matmul operands (RNN matmul dtype)
SMM_LAYOUT, SMM_COLS = _layout([
    ("xt", 12, 64),          # per-core x transposed, col = l*4+b
    ("a0t", 12, 16),
    ("ait", 16, 16),
    ("anT", 80, 256),        # [an_w[:,16:80].T ; an_w[:,0:16].T] rows
    # composite conv bias: rank-10 basis x 0/1 tap-validity patterns
    ("bcT", 10, 64),         # [cnn1_b ; W2_t @ cnn_b per tap] rows
    ("bcP", 10, 512),        # per-slot validity patterns (tiled x8 frames)
    # gate bias-into-psum operands: one matmul per psum tile
    # (lhsT = bias chunks as rows, rhs = chunk->column selector)
    ("brz4", 4, 128),        # (b_ih+b_hh) rz chunks
    ("bni4", 4, 128),        # [b_hh n chunks ; b_ih n chunks] rows
    ("fib2", 2, 128),        # fi_b chunks
    ("sel16", 4, 16),        # selector: col n lights chunk n//4
    ("selni", 4, 16),        # interleaved: even col j -> bhh j//8,
                             # odd col j -> bih j//8
    ("sel8", 2, 8),
])
# bias/affine tables (always fp32)
SMB_LAYOUT, SMB_COLS = _layout([
    ("pscale", 64, 1),       # inv/49/SC
    ("pshift", 64, 1),
    ("a0b", 16, 1),
    ("aib", 16, 1),
    ("anb", 128, 2),         # an_b chunks as cols
    ("fnb", 2, 1),
])

def build_nc():
    nc = bacc.Bacc("TRN2", target_bir_lowering=False, debug=False,
                   num_devices=NCORES)
    mm_rnn = _dt_of(MM_DT_RNN)
    MR = mm_rnn

    # big tensors are DMA'd bitcast to f32: the DMA engines are
    # element-rate limited, so 4x fewer elements = ~4x the bandwidth
    h_fr = nc.dram_tensor("fr", [NG, 128, 6 * FW // 4], F32,
                          kind="ExternalInput")
    h_smm = nc.dram_tensor("smm", [128, SMM_COLS], MR, kind="ExternalInput")
    h_smb = nc.dram_tensor("smb", [128, SMB_COLS], F32, kind="ExternalInput")
    h_wc = nc.dram_tensor("wc", [128, 864], F32, kind="ExternalInput")
    h_wih = nc.dram_tensor("wih", [128, 2 * 768], MR, kind="ExternalInput")
    h_whh = nc.dram_tensor("whh", [128, 2 * 768], MR, kind="ExternalInput")
    h_fi = nc.dram_tensor("fiw", [128, 2 * 256], MR, kind="ExternalInput")
    h_fn = nc.dram_tensor("fnw", [128, 4], MR, kind="ExternalInput")
    h_out = nc.dram_tensor("out", [2, 4 * HOR], F32, kind="ExternalOutput")

    def mm(out, lhsT, rhs, **kw):
        nc.tensor.matmul(out, lhsT, rhs, skip_group_check=True, **kw)

    with tile.TileContext(nc) as tc:
        from contextlib import ExitStack
        with ExitStack() as ctx:
            cpool = ctx.enter_context(tc.tile_pool(name="const", bufs=1))
            work = ctx.enter_context(tc.tile_pool(name="work", bufs=4))
            state = ctx.enter_context(tc.tile_pool(name="state", bufs=1))
            hpool = ctx.enter_context(tc.tile_pool(name="h", bufs=3))
            cps = ctx.enter_context(
                tc.tile_pool(name="cps", bufs=2, space="PSUM"))
            gps = ctx.enter_context(
                tc.tile_pool(name="gps", bufs=2, space="PSUM"))
            psr = ctx.enter_context(
                tc.tile_pool(name="psr", bufs=2, space="PSUM"))

            # ---- constants + frames to SBUF ----
            # Sync queue: g0 frames, conv weights, then the remaining
            # frame groups back-to-back (descriptor issue is ~0.6us each,
            # so one start per group, all resident). Scalar queue: the
            # small RNN weights, done well before first use.
            xins = []
            for g in range(NG):
                xin_g = cpool.tile([128, 6, FW], F8, tag=f"xin{g}")
                xins.append(xin_g)

            def fr_dma(g):
                nc.sync.dma_start(
                    xins[g][:].rearrange("p a b -> p (a b)").bitcast(F32),
                    h_fr[g])

            fr_dma(0)
            wc = cpool.tile([128, 3456], F8, tag="wc")
            nc.sync.dma_start(wc[:].bitcast(F32), h_wc[:])
            for g in range(1, NG):
                fr_dma(g)
            smm = cpool.tile([128, SMM_COLS], MR, tag="smm")
            nc.scalar.dma_start(smm[:], h_smm[:])
            smb = cpool.tile([128, SMB_COLS], F32, tag="smb")
            nc.scalar.dma_start(smb[:], h_smb[:])
            wih = cpool.tile([128, 2 * 768], MR, tag="wih")
            nc.scalar.dma_start(wih[:].bitcast(F32), h_wih[:].bitcast(F32))
            whh = cpool.tile([128, 2 * 768], MR, tag="whh")
            nc.scalar.dma_start(whh[:].bitcast(F32), h_whh[:].bitcast(F32))
            fiw = cpool.tile([128, 2 * 256], MR, tag="fiw")
            nc.scalar.dma_start(fiw[:].bitcast(F32), h_fi[:].bitcast(F32))
            fnw = cpool.tile([128, 4], MR, tag="fnw")
            nc.scalar.dma_start(fnw[:], h_fn[:])

            def sv(name):  # matmul-operand view (RNN dtype)
                r0, rows, off, width = SMM_LAYOUT[name]
                return smm[r0:r0 + rows, off:off + width]

            def svc(name, c0, w):
                r0, rows, off, width = SMM_LAYOUT[name]
                assert c0 + w <= width
                return smm[r0:r0 + rows, off + c0:off + c0 + w]

            def svf(name):  # fp32 bias/affine view
                r0, rows, off, width = SMB_LAYOUT[name]
                return smb[r0:r0 + rows, off:off + width]

            def svcf(name, c0, w):
                r0, rows, off, width = SMB_LAYOUT[name]
                assert c0 + w <= width
                return smb[r0:r0 + rows, off + c0:off + c0 + w]

            # PE warm-up: the tensor engine p-state ramps only under load,
            # and the first real matmuls otherwise run ~2.5x slow. Burn the
            # DMA-wait window (~2.5-10us) with throwaway matmuls on a
            # zeroed tile.
            wtile = work.tile([128, 512], F8, tag="warm")
            nc.vector.memset(wtile[:], 0.0)
            pw = psr.tile([64, 512], F32, tag="ps")
            for wi in range(16):
                mm(pw[:], wtile[:, 0:64], wtile[:, 0:512],
                   start=True, stop=True)

            # ---- persistent state tiles ----
            s2 = state.tile([16, 64], MR, tag="s2")
            s_enc = state.tile([128, 2, 64], MR, tag="senc")
            preds = state.tile([2, 4 * HOR], F32, tag="preds")
            # GRU scan operand tiles (even slots stay zero forever; odd
            # slots rewritten each step). Layout: slot 2i(+1) with pair
            # i = (chunk i//4, batch i%4) matching gate psum column order.
            rzscan = state.tile([128, 32], F32, tag="rzscan")
            nc.vector.memset(rzscan[:], 0.0)
            ozscan = state.tile([128, 16], F32, tag="ozscan")
            nc.gpsimd.memset(ozscan[:], 0.0)
            tzscan = state.tile([128, 16], F32, tag="tzscan")
            nc.gpsimd.memset(tzscan[:], 0.0)
            rz2 = rzscan[:].rearrange("p (a b) -> p a b", b=2)
            oz2 = ozscan[:].rearrange("p (a b) -> p a b", b=2)
            tz2 = tzscan[:].rearrange("p (a b) -> p a b", b=2)

            # ---- state adapters: s1 = relu(a0 x); s2 = s1 + relu(ai s1) ----
            # Emitted mid-group-0 so the conv1 matmuls (which only need
            # w1+xin0) lead the tensor queue instead of stalling on smm.
            def emit_adapters():
                pa = psr.tile([16, 64], F32, tag="ps")
                mm(pa[:], sv("a0t"), sv("xt"), start=True, stop=True)
                s1 = work.tile([16, 64], MR, tag="s1")
                nc.scalar.activation(s1[:], pa[:], AF.Relu, bias=svf("a0b"))
                pb = psr.tile([16, 64], F32, tag="ps")
                mm(pb[:], sv("ait"), s1[:], start=True, stop=True)
                s1b = work.tile([16, 64], MR, tag="s1")
                nc.scalar.activation(s1b[:], pb[:], AF.Relu, bias=svf("aib"))
                nc.vector.tensor_add(s2[:], s1[:], s1b[:])

            # force the combined activation table load at startup (hidden
            # in the DMA wait) instead of a 1.3us stall at the first
            # sigmoid: touch every function used, rarest first
            dum = work.tile([1, 4], F32, tag="dum")
            for fn in (AF.Sigmoid, AF.Tanh, AF.Relu, AF.Identity, AF.Copy):
                nc.scalar.activation(dum[:], rzscan[0:1, 0:4], fn)

            # encoder hidden state: odd slots of a scan-layout tile
            h0 = hpool.tile([128, 16], MR, tag="h")
            nc.gpsimd.memset(h0[:], 0.0)
            h_cur = h0[:].rearrange("p (a b) -> p a b", b=2)[:, :, 1]

            def whh_mms(prz16, pni16, hv):
                """whh gate matmuls for one step (the only mms after h)."""
                for mc in range(4):
                    reg = prz16[:, mc * 4:(mc + 1) * 4]
                    for kc in range(2):
                        mm(reg, whh[:, kc * 768 + mc * 128:
                                    kc * 768 + (mc + 1) * 128],
                           hv[:, kc * 4:(kc + 1) * 4],
                           start=False, stop=(mc == 3 and kc == 1))
                pniv = pni16.rearrange("p (c k) -> p c k", c=2)
                for mc2 in range(2):
                    reg = pniv[:, mc2, 0:8:2]
                    for kc in range(2):
                        mm(reg, whh[:, kc * 768 + (4 + mc2) * 128:
                                    kc * 768 + (5 + mc2) * 128],
                           hv[:, kc * 4:(kc + 1) * 4],
                           start=False, stop=(mc2 == 1 and kc == 1))

            def cell_chain(prz16, pni16, hv, after_sigma=None):
                """sigma -> scan(r*ghn+gin) -> tanh -> scan((1-z)n+zh).
                Returns the new hidden as an odd-slot view."""
                nc.scalar.activation(rz2[:, :, 1], prz16, AF.Sigmoid)
                if after_sigma is not None:
                    after_sigma()
                zv = rz2[:, 8:16, 1]
                nc.gpsimd.tensor_scalar(oz2[:, :, 1], zv, -1.0, 1.0,
                                        op0=ALU.mult, op1=ALU.add)
                nc.gpsimd.tensor_mul(tz2[:, :, 1], zv, hv)
                ns = work.tile([128, 16], F32, tag="nscan")
                nc.vector.tensor_tensor_scan(ns[:], rzscan[:, 0:16], pni16,
                                             0.0, op0=ALU.mult, op1=ALU.add)
                nc.scalar.activation(
                    tz2[:, :, 0],
                    ns[:].rearrange("p (a b) -> p a b", b=2)[:, :, 1],
                    AF.Tanh)
                hs = hpool.tile([128, 16], MR, tag="h")
                nc.vector.tensor_tensor_scan(hs[:], ozscan[:], tzscan[:],
                                             0.0, op0=ALU.mult, op1=ALU.add)
                return hs[:].rearrange("p (a b) -> p a b", b=2)[:, :, 1]

            def enc_preload(g):
                """Per-group gate psum tiles for steps 2g, 2g+1: biases +
                wih @ s_enc land before h is even known."""
                # start=True only on the bank's first mm: a start marks the
                # WHOLE psum bank pending-zero, so later first-writes of
                # other regions zero-fill implicitly (start=False).
                ep = gps.tile([128, 5, 16], F32, tag="eg")
                prz, pni = ep[:, 0:2, :], ep[:, 2:4, :]
                for ti in range(2):
                    mm(prz[:, ti, :], sv("brz4"), sv("sel16"),
                       start=(ti == 0), stop=False)
                    mm(pni[:, ti, :], sv("bni4"), sv("selni"),
                       start=False, stop=False)
                # pad write: clears the 16 elements past pni so CoreSim's
                # strided zero-region window never sees mixed state
                mm(ep[:, 4, :], sv("brz4"), sv("sel16"),
                   start=False, stop=True)
                pniv = pni.rearrange("p s (c k) -> p s c k", c=2)
                for ti in range(2):
                    xs = [s_enc[:, kc, g * FPG + ti * 4:g * FPG + ti * 4 + 4]
                          for kc in range(2)]
                    for mc in range(4):
                        reg = prz[:, ti, mc * 4:(mc + 1) * 4]
                        for kc in range(2):
                            mm(reg, wih[:, kc * 768 + mc * 128:
                                        kc * 768 + (mc + 1) * 128],
                               xs[kc], start=False, stop=False)
                    for mc2 in range(2):
                        reg = pniv[:, ti, mc2, 1:8:2]
                        for kc in range(2):
                            mm(reg, wih[:, kc * 768 + (4 + mc2) * 128:
                                        kc * 768 + (5 + mc2) * 128],
                               xs[kc], start=False,
                               stop=(mc2 == 1 and kc == 1))
                return prz, pni

            hstate = [h_cur]

            def group_cells(g):
                eprz, epni = enc_preload(g)
                for ti in range(2):
                    whh_mms(eprz[:, ti, :], epni[:, ti, :], hstate[0])
                    hstate[0] = cell_chain(eprz[:, ti, :], epni[:, ti, :],
                                           hstate[0])

            # adapters run in the conv-g0 DMA-wait window: smm lands
            # ~10us, well before the first conv matmul needs the PE
            emit_adapters()

            # ---- composite conv + features, per group of 8 frames ----
            for g in range(NG):
                xin = xins[g]
                pc = cps.tile([128, 2, 512], F32, tag="cps")
                P0, P1 = pc[:, 0, :], pc[:, 1, :]

                def rhs(off, q):
                    a = LEAD + off
                    return xin[:, 2 * q:2 * q + 2, a:a + 512]

                def wcv(blk, m):  # block at col 256*?: [128, 2, m]
                    return wc[:, blk:blk + 2 * m].rearrange(
                        "p (a m) -> p a m", a=2)

                # P0: first pair mm starts the accumulation over all 128
                # rows; bias + solo-tap (M=64) mms ride in the middle; the
                # last pair mm closes the group.
                for p, (lo, up) in enumerate(P0_PAIRS):
                    for q in range(3):
                        first = (p == 0 and q == 0)
                        last = (p == 2 and q == 2)
                        mm(P0, wcv((p * 3 + q) * 256, 128), rhs(lo, q),
                           start=first, stop=last, perf_mode=DR)
                        if first:
                            # conv bias (rank-10 basis x validity patterns)
                            mm(P0[0:64], sv("bcT"), sv("bcP"),
                               start=False, stop=False)
                            for q2 in range(3):
                                mm(P0[0:64], wcv(2304 + q2 * 128, 64),
                                   rhs(SOLO_TAP, q2),
                                   start=False, stop=False, perf_mode=DR)
                for q in range(3):
                    mm(P1, wcv(2688 + q * 256, 128), rhs(P1_PAIR[0], q),
                       start=(q == 0), stop=(q == 2), perf_mode=DR)

                # epilogue: combine shifted psum halves -> relu -> sum ->
                # affine. upper halves hold the paired tap accumulated at
                # slot+delta (P0 delta -8 = one grid row, P1 delta -1).
                p0g = pc[0:64, 0, :].rearrange("p (f a b) -> p f a b",
                                               a=8, b=8)
                p0u = pc[64:128, 0, :].rearrange("p (f a b) -> p f a b",
                                                 a=8, b=8)
                p1g = pc[0:64, 1, :].rearrange("p (f a b) -> p f a b",
                                               a=8, b=8)
                p1u = pc[64:128, 1, :].rearrange("p (f a b) -> p f a b",
                                                 a=8, b=8)
                # DVE/ACT ops cannot read two PSUM operands in one
                # instruction: stage the upper halves through SBUF.
                u0 = work.tile([64, 8, 7, 7], F32, tag="epu0")
                nc.scalar.activation(u0[:], p0u[:, :, 0:7, 1:8], AF.Copy)
                u1 = work.tile([64, 8, 7, 7], F32, tag="epu1")
                nc.scalar.activation(u1[:], p1u[:, :, 1:8, 0:7], AF.Copy)
                t0 = work.tile([64, 8, 7, 7], F32, tag="ep0")
                nc.vector.tensor_add(t0[:], p0g[:, :, 1:8, 1:8], u0[:])
                t1 = work.tile([64, 8, 7, 7], F32, tag="ep1")
                nc.vector.tensor_add(t1[:], p1g[:, :, 1:8, 1:8], u1[:])
                t2 = work.tile([64, 8, 7, 7], F32, tag="ep2")
                nc.vector.tensor_add(t2[:], t0[:], t1[:])
                t3 = work.tile([64, 8, 7, 7], F32, tag="ep")
                nc.vector.tensor_scalar_max(t3[:], t2[:], 0.0)
                red = work.tile([64, 8], F32, tag="red")
                nc.vector.tensor_reduce(red[:], t3[:],
                                        axis=mybir.AxisListType.XY,
                                        op=ALU.add)
                feats = work.tile([80, 8], MR, tag="feats")
                nc.scalar.activation(feats[0:64, :], red[:], AF.Identity,
                                     bias=svf("pshift"), scale=svf("pscale"))

                # an: relu(an_w [s2; feats] + an_b), one K=80 matmul per half
                gcol = slice(g * FPG, (g + 1) * FPG)
                nc.gpsimd.tensor_copy(feats[64:80, :], s2[:, gcol])
                for mc in range(2):
                    pan = psr.tile([128, FPG], F32, tag="ps")
                    mm(pan[:], svc("anT", mc * 128, 128), feats[:],
                       start=True, stop=True)
                    nc.scalar.activation(s_enc[:, mc, gcol], pan[:], AF.Relu,
                                         bias=svcf("anb", mc, 1))

                # encoder steps, emitted TWO groups late: their chain-
                # blocked ops would otherwise park in the 4-deep engine
                # wait queues and head-block later groups' epilogue work
                if g >= 2:
                    group_cells(g - 2)

            for g in (NG - 2, NG - 1):
                group_cells(g)

            # ---- decoder ----
            # Emission order puts everything that depends only on hn(t-1)
            # (whh parts) ahead of the xr(t-1)-dependent wih work, and
            # defers fn(t-1) behind the whh block, so the tensor queue
            # keeps moving during the fi/xr window.
            def emit_fn(x, tt):
                pfn = psr.tile([2, 4], F32, tag="ps")
                for kc in range(2):
                    mm(pfn[:], fnw[:, kc * 2:(kc + 1) * 2],
                       x[:, kc * 4:(kc + 1) * 4],
                       start=(kc == 0), stop=(kc == 1))
                nc.scalar.activation(preds[:, tt * 4:(tt + 1) * 4], pfn[:],
                                     AF.Tanh, bias=svf("fnb"))

            xi, hh = hstate[0], hstate[0]
            for t in range(HOR):
                dp = gps.tile([128, 5, 16], F32, tag="eg")
                prz, pni = dp[:, 0, :], dp[:, 1, :]
                mm(prz, sv("brz4"), sv("sel16"), start=True, stop=False)
                mm(pni, sv("bni4"), sv("selni"), start=False, stop=False)
                pniv = pni.rearrange("p (c k) -> p c k", c=2)
                # whh parts (ready at hh)
                for mc in range(4):
                    reg = prz[:, mc * 4:(mc + 1) * 4]
                    for kc in range(2):
                        mm(reg, whh[:, kc * 768 + mc * 128:
                                    kc * 768 + (mc + 1) * 128],
                           hh[:, kc * 4:(kc + 1) * 4],
                           start=False, stop=False)
                for mc2 in range(2):
                    reg = pniv[:, mc2, 0:8:2]
                    for kc in range(2):
                        mm(reg, whh[:, kc * 768 + (4 + mc2) * 128:
                                    kc * 768 + (5 + mc2) * 128],
                           hh[:, kc * 4:(kc + 1) * 4],
                           start=False, stop=False)
                if t > 0:
                    emit_fn(xi, t - 1)
                # wih parts (ready at xi)
                mm(dp[:, 2, :], sv("brz4"), sv("sel16"),
                   start=False, stop=True)
                for mc in range(4):
                    reg = prz[:, mc * 4:(mc + 1) * 4]
                    for kc in range(2):
                        mm(reg, wih[:, kc * 768 + mc * 128:
                                    kc * 768 + (mc + 1) * 128],
                           xi[:, kc * 4:(kc + 1) * 4],
                           start=False, stop=(mc == 3 and kc == 1))
                for mc2 in range(2):
                    reg = pniv[:, mc2, 1:8:2]
                    for kc in range(2):
                        mm(reg, wih[:, kc * 768 + (4 + mc2) * 128:
                                    kc * 768 + (5 + mc2) * 128],
                           xi[:, kc * 4:(kc + 1) * 4],
                           start=False, stop=(mc2 == 1 and kc == 1))
                # fi bias rides the idle tensor window before hn is ready
                pfi = psr.tile([128, 8], F32, tag="ps")
                mm(pfi[:], sv("fib2"), sv("sel8"), start=True, stop=False)

                hn = cell_chain(prz, pni, hh)

                # final_i residual: xr = hn + relu(fi hn + fi_b)
                for mc2 in range(2):
                    reg = pfi[:, mc2 * 4:(mc2 + 1) * 4]
                    for kc2 in range(2):
                        mm(reg, fiw[:, kc2 * 256 + mc2 * 128:
                                    kc2 * 256 + (mc2 + 1) * 128],
                           hn[:, kc2 * 4:(kc2 + 1) * 4],
                           start=False, stop=(mc2 == 1 and kc2 == 1))
                xr = hpool.tile([128, 8], MR, tag="xr")
                nc.vector.scalar_tensor_tensor(
                    xr[:], pfi[:], 0.0, hn,
                    op0=ALU.max, op1=ALU.add)
                xi, hh = xr[:], hn
            emit_fn(xi, HOR - 1)

            nc.sync.dma_start(h_out[:], preds[:])

    nc.finalize()
    return nc


# ---------------- host-side data prep ----------------

def _prep_frames(frames):
    """frames (32,16,3,112,112) -> per-core [NG, 128, 6*FW] gutter-layout
    patch-T fp8 (8x8 cell grid per frame, row0/col0 + LEAD/TAIL zeros)."""
    out = np.empty((NCORES, NG, 128, 6 * FW), mybir.dt.np(F8))
    fr = np.ascontiguousarray(frames, np.float32)
    for c in range(NCORES):
        fb = fr[c * BPC:(c + 1) * BPC]  # (4, 16, 3, 112, 112)
        a = fb.reshape(BPC, L, 3, 7, 16, 7, 16)
        # -> [l, b, ch, kh, kw, ph, pw]
        a = a.transpose(1, 0, 2, 4, 6, 3, 5)
        a = a.reshape(L, BPC, 768, 49)
        a = a.reshape(NG, 2, BPC, 6, 128, 49)
        # -> [g, k, p, li, b, s]
        a = a.transpose(0, 3, 4, 1, 2, 5)
        a = a.reshape(NG, 6, 128, 8, 7, 7)
        buf = np.zeros((NG, 6, 128, FW), np.float32)
        grid = buf[:, :, :, LEAD:LEAD + 512].reshape(NG, 6, 128, 8, 8, 8)
        grid[:, :, :, :, 1:8, 1:8] = a
        out[c] = buf.transpose(0, 2, 1, 3).reshape(
            NG, 128, 6 * FW).astype(mybir.dt.np(F8))
    return out


def _tap_weights(iv):
    """Composite per-tap weights W_t = cnn1_w[:,:,dh,dw] @ W1 (64, 768),
    keyed by gutter-slot offset (dh-1)*8 + (dw-1), scaled by SC."""
    W1f = iv["cnn_w"].reshape(576, 768).astype(np.float32)
    T = {}
    for dh in range(3):
        for dw in range(3):
            off = (dh - 1) * 8 + (dw - 1)
            T[off] = (iv["cnn1_w"][:, :, dh, dw].astype(np.float32)
                      @ W1f) * SC
    return T


def _prep_weights(iv):
    w = {}
    f8 = mybir.dt.np(F8)
    T = _tap_weights(iv)

    # composite conv lhsT blocks, in matmul emission order:
    # 9x [128, 2, 128] P0 pair blocks, 3x [128, 2, 64] solo-tap blocks,
    # 3x [128, 2, 128] P1 pair blocks.
    wcb = np.zeros((128, 3456), np.float32)
    col = 0
    for pair in P0_PAIRS + [None, P1_PAIR]:
        for q in range(3):
            if pair is None:
                blk = np.zeros((128, 2, 64), np.float32)
                for j in range(2):
                    c = 2 * q + j
                    blk[:, j, :] = T[SOLO_TAP][:, c * 128:(c + 1) * 128].T
                wcb[:, col:col + 128] = blk.reshape(128, 128)
                col += 128
            else:
                lo, up = pair
                blk = np.zeros((128, 2, 128), np.float32)
                for j in range(2):
                    c = 2 * q + j
                    blk[:, j, 0:64] = T[lo][:, c * 128:(c + 1) * 128].T
                    blk[:, j, 64:128] = T[up][:, c * 128:(c + 1) * 128].T
                wcb[:, col:col + 256] = blk.reshape(128, 256)
                col += 256
    assert col == 3456
    w["wc"] = np.ascontiguousarray(wcb).astype(f8).view(np.float32)

    rdt = mybir.dt.np(_dt_of(MM_DT_RNN))
    for name, key in (("wih", "w_ih"), ("whh", "w_hh")):
        T = iv[key].T.astype(np.float32)  # (256, 768)
        w[name] = np.ascontiguousarray(
            T.reshape(2, 128, 768).transpose(1, 0, 2).reshape(
                128, 1536)).astype(rdt)
    T = iv["fi_w"].T.astype(np.float32)  # (256, 256)
    w["fiw"] = np.ascontiguousarray(
        T.reshape(2, 128, 256).transpose(1, 0, 2).reshape(128, 512)).astype(rdt)
    T = iv["fn_w"].T.astype(np.float32)  # (256, 2)
    w["fnw"] = np.ascontiguousarray(
        T.reshape(2, 128, 2).transpose(1, 0, 2).reshape(128, 4)).astype(rdt)
    return w


def _prep_smalls(iv, x, core):
    smm = np.zeros((128, SMM_COLS), mybir.dt.np(_dt_of(MM_DT_RNN)))
    smb = np.zeros((128, SMB_COLS), np.float32)

    def put(name, arr):
        if name in SMM_LAYOUT:
            r0, rows, off, width = SMM_LAYOUT[name]
            dst = smm
        else:
            r0, rows, off, width = SMB_LAYOUT[name]
            dst = smb
        a = np.asarray(arr, np.float32).reshape(rows, width)
        dst[r0:r0 + rows, off:off + width] = a.astype(dst.dtype)

    # composite conv bias = cnn1_b + sum over in-range taps of
    # (W2_t @ cnn_b): rank-10 basis (bcT) x 0/1 validity patterns (bcP),
    # folded into PSUM by one matmul; x SC to match the psum scale
    M = np.einsum("oiab,i->oab", iv["cnn1_w"], iv["cnn_b"]).astype(np.float32)
    bct = np.zeros((10, 64), np.float32)
    bcp = np.zeros((10, 512), np.float32)
    bct[0] = iv["cnn1_b"].astype(np.float32)
    bcp[0] = 1.0
    grid = bcp.reshape(10, 8, 8, 8)
    ti = 1
    for dh in range(3):
        for dw in range(3):
            bct[ti] = M[:, dh, dw]
            for r in range(8):
                for cc in range(8):
                    if 2 <= r + dh <= 8 and 2 <= cc + dw <= 8:
                        grid[ti, :, r, cc] = 1.0
            ti += 1
    put("bcT", bct * SC)
    put("bcP", bcp)

    inv = iv["bn_g"] / np.sqrt(iv["bn_v"] + BN_EPS)
    put("pscale", (inv / 49.0 / SC)[:, None])
    put("pshift", (iv["bn_b"] - iv["bn_m"] * inv)[:, None])

    xb = x[core * BPC:(core + 1) * BPC]  # (4, 16, 12)
    put("xt", xb.transpose(2, 1, 0).reshape(12, 64))

    put("a0t", iv["a0_w"].T)
    put("a0b", iv["a0_b"][:, None])
    put("ait", iv["ai_w"].T)
    put("aib", iv["ai_b"][:, None])
    put("anT", np.concatenate([iv["an_w"][:, 16:80].T,
                               iv["an_w"][:, 0:16].T], axis=0))
    put("anb", iv["an_b"].reshape(2, 128).T)

    put("fnb", iv["fn_b"][:, None])
    bs = (iv["b_ih"] + iv["b_hh"]).astype(np.float32)
    put("brz4", bs[:512].reshape(4, 128))
    # n-gate interleaved bias: rows [bhh c0, bhh c1, bih c0, bih c1];
    # slot j = c*8 + b*2 + parity -> row parity*2 + c
    put("bni4", np.concatenate([iv["b_hh"][512:].reshape(2, 128),
                                iv["b_ih"][512:].reshape(2, 128)]))
    selni = np.zeros((4, 16), np.float32)
    for j in range(16):
        selni[(j % 2) * 2 + j // 8, j] = 1.0
    put("selni", selni)
    put("fib2", iv["fi_b"].reshape(2, 128))
    put("sel16", np.repeat(np.eye(4, dtype=np.float32), 4, axis=1))
    put("sel8", np.repeat(np.eye(2, dtype=np.float32), 4, axis=1))
    return smm, smb


def make_in_maps(inputs):
    iv = {k: np.asarray(v, np.float32) for k, v in inputs.items()}
    frames = iv["frames"]
    x = iv["x"]
    fr_all = _prep_frames(frames)
    w = _prep_weights(iv)
    in_maps = []
    for c in range(NCORES):
        smm, smb = _prep_smalls(iv, x, c)
        m = {"fr": np.ascontiguousarray(fr_all[c]).view(np.float32),
             "smm": smm, "smb": smb}
        m.update(w)
        in_maps.append(m)
    return in_maps


_NC_CACHE = None


def get_nc():
    global _NC_CACHE
    if _NC_CACHE is None:
        _NC_CACHE = build_nc()
    return _NC_CACHE


def _install_ntff_hook():
    """The agent image's antenv lacks axon_hooks; synthesize it so
    run_bass_kernel_spmd(trace=True) can capture NTFF profiles."""
    try:
        from antenv.axon_hooks import get_axon_ntff_profile_hook  # noqa: F401
        return True
    except ImportError:
        pass
    try:
        import types
        import antenv
        if "/root/.axon_site" not in sys.path:
            sys.path.insert(0, "/root/.axon_site")
        from trn_agent_boot.trn_boot import _ntff_profile_via_ctypes
        hook = _ntff_profile_via_ctypes("/opt/axon/libaxon_pjrt.so")
        mod = types.ModuleType("antenv.axon_hooks")
        mod.get_axon_ntff_profile_hook = lambda: hook
        mod.set_axon_ntff_profile_hook = lambda h: None
        sys.modules["antenv.axon_hooks"] = mod
        antenv.axon_hooks = mod
        return hook is not None
    except Exception as e:  # pragma: no cover - profiling is best-effort
        print(f"ntff hook install failed: {e}")
        return False


def kernel(**inputs):
    global LAST_EXEC_NS, LAST_RESULTS
    nc = get_nc()
    in_maps = make_in_maps(inputs)
    trace = bool(int(os.environ.get("KERNEL_TRACE", "0")))
    if trace:
        trace = _install_ntff_hook()
    res = run_bass_kernel_spmd(nc, in_maps, core_ids=list(range(NCORES)),
                               trace=trace)
    LAST_RESULTS = res
    LAST_EXEC_NS = res.exec_time_ns
    outs = []
    for c in range(NCORES):
        o = res.results[c]["out"]  # (2, 40)
        outs.append(o.reshape(2, HOR, BPC).transpose(1, 2, 0)[:, :, None, :])
    return np.concatenate(outs, axis=1).astype(np.float32)


if __name__ == "__main__":
    nc = get_nc()
    print("built ok; instructions:",
          sum(len(bb.instructions) for bb in nc.main_func.blocks))

